# revision 1
# baseline (speedup 1.0000x reference)
"""Trainium2 Bass kernel for CondLaneRNNHead-style dynamic mask head.

Computation (see reference): per-instance 3-layer 1x1-conv MLP over
per-image feature maps augmented with 2 coordinate channels.

  out[m] = w2[m] @ relu(w1[m] @ relu(w0[m] @ [coords; x[img(m)]] + b0[m]) + b1[m]) + b2[m]

Shapes: x [4, 64, 80, 200] f32, mask_head_params [32, 8513] f32, num_ins=8.
Output [1, 32, 80, 200] f32.

Sharding: spatial, along H. Core k processes rows [10k, 10k+10) of all 4
images for all 32 instances. This replicates only the (small) per-instance
params across cores; the big x tensor is read exactly once in aggregate.

Device kernel structure (per core):
  - feats[img] SBUF tile [66, 2000]: partitions 0-63 = x channels,
    partitions 64-65 = (xx/W, yy/W) coordinate rows.
  - Instances are packed in PAIRS (2 instances of the same image):
      layer0: lhsT [66, 128]  (cols 0-63 inst a, 64-127 inst b), one matmul
              computes both instances' 64 hidden channels.
      layer1: lhsT [128, 128] block-diagonal (w1a.T | w1b.T).
      layer2: lhsT [128, 2]   ([w2a;0] | [0;w2b]).
    Matmuls run in bf16 (fp32 PSUM accumulate). fp32r was measured 5x
    slower: fp32 weights get no fast-weight-load and no background weight
    buffer, so every matmul serialized a ~300ns LDWEIGHTS and paid isolated
    fill+drain latency (~880ns/matmul vs ~220ns bf16 warm).
  - ReLU+bias layer0 on ScalarE (activation, PSUM->SBUF); layer1 on VectorE
    (tensor_scalar add+max, PSUM->SBUF) to balance engines.
  - layer2 outputs [2, free] of 4 pairs (a "quad" = 8 instances, one image)
    are packed into ONE PSUM tile at partition offsets {0,32,64,96} (matmul
    col tile positions), so the PSUM->SBUF move + b2 bias is a single
    [128, 2000] op per quad instead of 16 thin [2, .] ops. DMA cannot read
    PSUM on trn2, and ACT/DVE op cost is free-dim-driven (partition count
    free), so dense partition packing is what makes the move cheap.
"""

import numpy as np
from contextlib import ExitStack

N_IMG, C, H, W = 4, 64, 80, 200
NUM_INS = 8
M = N_IMG * NUM_INS          # 32 instances
N_CORES = 8
HPC = H // N_CORES           # 10 rows of H per core
SPI = HPC * W                # 2000 spatial positions per image slice
PAIRS = M // 2               # 16
CH = C + 2                   # 66 input channels incl. coords
FD = 1000                    # activation chunk
# matmul free-dim splits inside each 1000 chunk: PSUM banks hold 512 f32, and
# a matmul output must not cross a bank boundary -> split 512 + 488.
SPLITS = ((0, 512), (512, 488))
FDP = 1024                   # padded per-half stride in the quad PSUM tile

_W0N, _W1N, _W2N = CH * C, C * C, C
_B2_SHIFT = -2.19

_COMPILED = {}


def _build_program():
    import concourse.bacc as bacc
    import concourse.tile as tile
    from concourse import mybir

    dt = mybir.dt
    AF = mybir.ActivationFunctionType
    OP = mybir.AluOpType

    nc = bacc.Bacc("TRN2", target_bir_lowering=False, debug=False)

    # xs packs the 2 coordinate rows below the 64 x-channels so each image's
    # feats tile is filled by a single DMA (matmuls tolerate few sync waits).
    xs_d = nc.dram_tensor("xs", [N_IMG, CH, SPI], dt.bfloat16, kind="ExternalInput").ap()
    # layer0 lhsT zero-padded to K=128: K=66 matmuls light up only half the
    # PE rows, which keeps the HAM activity monitor below its un-throttle
    # threshold (PE then runs at 1.2 instead of 2.4 GHz).
    l0_d = nc.dram_tensor("l0t", [128, PAIRS * 128], dt.bfloat16, kind="ExternalInput").ap()
    l1_d = nc.dram_tensor("l1t", [128, PAIRS * 128], dt.bfloat16, kind="ExternalInput").ap()
    # layer2 runs in bf16: fp32r matmuls require dst start_partition == 0,
    # which the quad partition-packing (offsets 32/64/96) violates.
    # lhsT cols 2-31 are zeros: each mm2 then writes a full 32-partition
    # group, keeping PSUM fully initialized at no PE cost (time ~ free size).
    l2_d = nc.dram_tensor("l2t", [128, PAIRS * 32], dt.bfloat16, kind="ExternalInput").ap()
    b0_d = nc.dram_tensor("b0t", [128, PAIRS], dt.float32, kind="ExternalInput").ap()
    b1_d = nc.dram_tensor("b1t", [128, PAIRS], dt.float32, kind="ExternalInput").ap()
    b2_d = nc.dram_tensor("b2q", [128, 4], dt.float32, kind="ExternalInput").ap()
    # out[q, j, r, :] = instance 8q + 2j + r, i.e. plain instance-major order
    out_d = nc.dram_tensor("out", [4, 4, 2, SPI], dt.float32, kind="ExternalOutput").ap()

    f32 = dt.float32
    bf16 = dt.bfloat16

    with tile.TileContext(nc) as tc, ExitStack() as ctx:
        cpool = ctx.enter_context(tc.tile_pool(name="const", bufs=1))
        hpool = ctx.enter_context(tc.tile_pool(name="work", bufs=4))
        pspool = ctx.enter_context(tc.tile_pool(name="ps", bufs=3, space="PSUM"))
        psqpool = ctx.enter_context(tc.tile_pool(name="psq", bufs=1, space="PSUM"))

        # ---- resident tiles + loads ----
        # Small bias/weight tensors first: the sync sequencer issues DMAs in
        # order (~0.6us each), and the first ReLU needs b0s — emitting biases
        # last cost a ~20us pipeline stall at the head.
        b0s = cpool.tile([128, PAIRS], f32, tag="b0s", name="b0s")
        nc.sync.dma_start(b0s[:], b0_d[:])
        b1s = cpool.tile([128, PAIRS], f32, tag="b1s", name="b1s")
        nc.sync.dma_start(b1s[:], b1_d[:])
        b2s = cpool.tile([128, 4], f32, tag="b2s", name="b2s")
        nc.sync.dma_start(b2s[:], b2_d[:])
        l2s = cpool.tile([128, PAIRS * 32], bf16, tag="l2s", name="l2s")
        nc.sync.dma_start(l2s[:], l2_d[:])
        fe = []
        l0s = []
        l1s = []
        for n in range(N_IMG):
            t = cpool.tile([128, SPI], bf16, tag=f"fe{n}", name=f"fe{n}")
            # memset first (engines need 32-aligned partition starts); the
            # coord-row DMA below then overwrites rows 64-65
            nc.gpsimd.memset(t[64:128, :], 0.0)
            # split across partition chunks so the transfer spreads over
            # multiple DMA engines (~23 GB/s each)
            for a, b in ((0, 17), (17, 34), (34, 50), (50, CH)):
                nc.sync.dma_start(t[a:b, :], xs_d[n, a:b, :])
            fe.append(t)
            g0 = cpool.tile([128, 4 * 128], bf16, tag=f"l0g{n}", name=f"l0g{n}")
            nc.sync.dma_start(g0[:], l0_d[:, n * 512 : (n + 1) * 512])
            l0s.append(g0)
            g1 = cpool.tile([128, 4 * 128], bf16, tag=f"l1g{n}", name=f"l1g{n}")
            nc.sync.dma_start(g1[:], l1_d[:, n * 512 : (n + 1) * 512])
            l1s.append(g1)

        # ---- PE warmup on a zeroed dummy tile ----
        # Runs during the input-DMA head (no data deps), attempting to lift
        # the HAM clock gate (1.2 -> 2.4 GHz) before the real stream; costs
        # nothing even if the gate stays cold since it overlaps the DMAs.
        wsrc = cpool.tile([128, 640], bf16, tag="wsrc", name="wsrc")
        nc.gpsimd.memset(wsrc[:], 0.0)
        # dummy Relu so the ACT table-set DMA issues at t~0 instead of
        # queueing behind the input DMAs (measured 22us first-ReLU stall)
        wact = cpool.tile([128, 8], f32, tag="wact", name="wact")
        nc.scalar.activation(wact[:], wsrc[:, 0:8], AF.Relu, bias=0.0)
        wps = pspool.tile([128, FD], f32, tag="ps", name="wps")
        for _ in range(44):
            nc.tensor.matmul(
                wps[:, 0:512], wsrc[:, 0:128], wsrc[:, 128:640],
                start=True, stop=True,
            )

        # ---- main loop: quads of pairs (8 instances of one image) ----
        for q in range(4):
            img = q
            for hh in range(SPI // FD):
                base = hh * FD
                psq = psqpool.tile([128, FDP], f32, tag="psq", name="psq")
                # during the pipeline ramp (first chains in flight) the PE
                # sits idle between a pair's layers; in-order execution means
                # only instructions placed THERE can fill the gap. These
                # zero-matmuls stomp a freshly allocated PSUM tile BEFORE its
                # real start=True matmuls overwrite it, keeping HAM activity
                # up so the 2.4 GHz clock survives the ramp.
                def _fill(dst, n_mm):
                    for _ in range(n_mm):
                        nc.tensor.matmul(
                            dst[:, 0:512], wsrc[:, 0:128], wsrc[:, 128:640],
                            start=True, stop=True,
                        )

                ramp = q == 0 and hh == 0
                for j in range(4):
                    p = 4 * q + j
                    w0 = l0s[img][:, j * 128 : (j + 1) * 128]
                    w1 = l1s[img][:, j * 128 : (j + 1) * 128]
                    w2 = l2s[:, 32 * p : 32 * p + 32]
                    ps0 = pspool.tile([128, FD], f32, tag="ps", name="ps0")
                    if ramp and j < 3:
                        _fill(ps0, 4 - j)
                    for off, sz in SPLITS:
                        nc.tensor.matmul(
                            ps0[:, off : off + sz],
                            w0,
                            fe[img][:, base + off : base + off + sz],
                            start=True,
                            stop=True,
                        )
                    h1 = hpool.tile([128, FD], bf16, tag="h1", name="h1")
                    if p % 2 == 0:
                        nc.scalar.activation(
                            h1[:], ps0[:], AF.Relu, bias=b0s[:, p : p + 1]
                        )
                    else:
                        nc.vector.tensor_scalar(
                            h1[:], ps0[:], b0s[:, p : p + 1], 0.0, OP.add, OP.max
                        )
                    ps1 = pspool.tile([128, FD], f32, tag="ps", name="ps1")
                    if ramp and j < 3:
                        _fill(ps1, 4 - j)
                    for off, sz in SPLITS:
                        nc.tensor.matmul(
                            ps1[:, off : off + sz],
                            w1,
                            h1[:, off : off + sz],
                            start=True,
                            stop=True,
                        )
                    h2 = hpool.tile([128, FD], bf16, tag="h2", name="h2")
                    if p % 2 == 0:
                        nc.vector.tensor_scalar(
                            h2[:], ps1[:], b1s[:, p : p + 1], 0.0, OP.add, OP.max
                        )
                    else:
                        nc.scalar.activation(
                            h2[:], ps1[:], AF.Relu, bias=b1s[:, p : p + 1]
                        )
                    for off, sz in SPLITS:
                        nc.tensor.matmul(
                            psq[32 * j : 32 * j + 32, off : off + sz],
                            w2,
                            h2[:, off : off + sz],
                            start=True,
                            stop=True,
                            tile_position=(0, 32 * j),
                        )
                # fused bias+move for the (quad, half); alternate engines
                oq = hpool.tile([128, FD], f32, tag="oq", name="oq", bufs=6)
                if (2 * q + hh) % 2 == 0:
                    nc.scalar.activation(
                        oq[:], psq[:, 0:FD], AF.Identity, bias=b2s[:, q : q + 1]
                    )
                else:
                    nc.vector.tensor_scalar(
                        oq[:], psq[:, 0:FD], b2s[:, q : q + 1], None, OP.add
                    )
                # contiguous 2-row reads (strided-partition reads miss deps
                # in the tile tracker and raced the move op)
                # issue output DMAs from the otherwise-idle gpsimd queue so
                # they don't serialize behind input DMAs on the sync engine
                for j in range(4):
                    nc.gpsimd.dma_start(
                        out_d[q, j, :, base : base + FD], oq[32 * j : 32 * j + 2, :]
                    )

    nc.compile()
    _dedupe_ldweights(nc, mybir)
    return nc


def _dedupe_ldweights(nc, mybir):
    """Drop redundant PE LDWEIGHTS after compile.

    Tile emits one LDWEIGHTS per matmul; consecutive matmuls here often share
    one stationary operand (split-column pairs, the warmup burst), so the
    repeat loads only serialize the PE (~120ns each, and they block drain/fill
    overlap between back-to-back matmuls). Safe removal criteria: identical
    weights AP + tile_position as the last retained LDWEIGHTS (nothing between
    two LDWEIGHTS changes the loaded weights), and no semaphore waits/updates
    on the dropped instruction, so synchronization is untouched.
    """
    dropped = 0
    for fn in nc.m.functions:
        for blk in fn.blocks:
            new = []
            last_sig = None
            for i in blk.instructions:
                if (
                    isinstance(i, mybir.InstLdweights)
                    and i.engine == mybir.EngineType.PE
                ):
                    sig = (
                        str(i.ins[0]),
                        tuple(i.tile_position or ()),
                        i.perf_mode,
                        i.is_transpose,
                    )
                    si = i.sync_info
                    clean = si is None or (not si.on_wait and not si.on_update)
                    if clean and sig == last_sig:
                        dropped += 1
                        continue
                    last_sig = sig
                new.append(i)
            if dropped:
                blk.instructions.clear()
                blk.instructions.extend(new)
    return dropped


def _pack_params(mask_head_params):
    """Split generated params and build the pair-packed device layouts."""
    p = np.ascontiguousarray(mask_head_params, dtype=np.float32)
    o0, o1, o2 = _W0N, _W0N + _W1N, _W0N + _W1N + _W2N
    w0 = p[:, :o0].reshape(M, C, CH)
    w1 = p[:, o0:o1].reshape(M, C, C)
    w2 = p[:, o1:o2].reshape(M, C)
    b0 = p[:, o2 : o2 + C]
    b1 = p[:, o2 + C : o2 + 2 * C]
    b2 = p[:, o2 + 2 * C :] + np.float32(_B2_SHIFT)

    import ml_dtypes as _mld

    # layer0 lhsT [128, 16*128]: rows 0-63 = x-channel weights, 64-65 = coord
    # weights, 66-127 zero K-pad; cols pair-major then (inst a | inst b).
    w0T = np.transpose(w0, (2, 0, 1))              # [66(cin), 32, 64]
    w0T = np.concatenate([w0T[2:], w0T[:2]], 0)    # x channels first, coords last
    l0t = np.zeros((128, M * C), dtype=np.float32)
    l0t[:CH] = w0T.reshape(CH, M * C)
    l0t = np.ascontiguousarray(l0t.astype(_mld.bfloat16))

    l1 = np.zeros((PAIRS, 128, 128), dtype=np.float32)
    l1[:, :C, :C] = np.transpose(w1[0::2], (0, 2, 1))
    l1[:, C:, C:] = np.transpose(w1[1::2], (0, 2, 1))
    l1t = np.ascontiguousarray(
        np.transpose(l1, (1, 0, 2)).reshape(128, PAIRS * 128).astype(_mld.bfloat16)
    )

    import ml_dtypes
    l2 = np.zeros((PAIRS, 128, 32), dtype=np.float32)
    l2[:, :C, 0] = w2[0::2]
    l2[:, C:, 1] = w2[1::2]
    l2t = np.ascontiguousarray(
        np.transpose(l2, (1, 0, 2)).reshape(128, PAIRS * 32).astype(ml_dtypes.bfloat16)
    )

    b0t = np.ascontiguousarray(np.concatenate([b0[0::2], b0[1::2]], 1).T)  # [128,16]
    b1t = np.ascontiguousarray(np.concatenate([b1[0::2], b1[1::2]], 1).T)
    # b2 packed to match the quad PSUM layout: rows 32j+r of col q hold
    # instance 8q + 2j + r.
    b2q = np.zeros((128, 4), dtype=np.float32)
    for qq in range(4):
        for j in range(4):
            b2q[32 * j, qq] = b2[8 * qq + 2 * j, 0]
            b2q[32 * j + 1, qq] = b2[8 * qq + 2 * j + 1, 0]
    return l0t, l1t, l2t, b0t, b1t, b2q


def _run(x, mask_head_params, trace=False, trace_kwargs=None):
    from concourse.bass_utils import run_bass_kernel_spmd

    if "nc" not in _COMPILED:
        _COMPILED["nc"] = _build_program()
    nc = _COMPILED["nc"]

    x = np.ascontiguousarray(x, dtype=np.float32)
    l0t, l1t, l2t, b0t, b1t, b2q = _pack_params(mask_head_params)

    xx = np.tile(np.arange(W, dtype=np.float32) / W, HPC)  # [2000]
    in_maps = []
    for k in range(N_CORES):
        h0 = k * HPC
        yy = np.repeat((h0 + np.arange(HPC, dtype=np.float32)) / W, W)
        coords = np.stack([xx, yy], 0)  # [2, 2000]
        import ml_dtypes as _mld

        xsl = x[:, :, h0 : h0 + HPC, :].reshape(N_IMG, C, SPI)
        xs = np.ascontiguousarray(
            np.concatenate(
                [xsl, np.broadcast_to(coords, (N_IMG, 2, SPI))], axis=1
            ).astype(_mld.bfloat16)
        )
        in_maps.append(
            {
                "xs": xs,
                "l0t": l0t,
                "l1t": l1t,
                "l2t": l2t,
                "b0t": b0t,
                "b1t": b1t,
                "b2q": b2q,
            }
        )

    res = run_bass_kernel_spmd(
        nc,
        in_maps,
        list(range(N_CORES)),
        trace=trace,
        **(trace_kwargs or {}),
    )

    out = np.empty((1, M, H, W), dtype=np.float32)
    for k in range(N_CORES):
        oc = res.results[k]["out"].reshape(M, HPC, W)
        out[0, :, k * HPC : (k + 1) * HPC, :] = oc
    return out, res


def kernel(x, mask_head_params, num_ins):
    n_ins = int(np.asarray(num_ins))
    assert n_ins == NUM_INS, f"kernel hardcoded for num_ins={NUM_INS}, got {n_ins}"
    out, _ = _run(x, mask_head_params)
    return out



# revision 2
# speedup vs baseline: 1.0337x; 1.0337x over previous
"""Trainium2 Bass kernel for CondLaneRNNHead-style dynamic mask head.

Computation (see reference): per-instance 3-layer 1x1-conv MLP over
per-image feature maps augmented with 2 coordinate channels.

  out[m] = w2[m] @ relu(w1[m] @ relu(w0[m] @ [coords; x[img(m)]] + b0[m]) + b1[m]) + b2[m]

Shapes: x [4, 64, 80, 200] f32, mask_head_params [32, 8513] f32, num_ins=8.
Output [1, 32, 80, 200] f32.

Sharding: spatial, along H. Core k processes rows [10k, 10k+10) of all 4
images for all 32 instances. This replicates only the (small) per-instance
params across cores; the big x tensor is read exactly once in aggregate.

Device kernel structure (per core):
  - feats[img] SBUF tile [66, 2000]: partitions 0-63 = x channels,
    partitions 64-65 = (xx/W, yy/W) coordinate rows. Layer0 matmuls run
    K=66 directly (no zero-pad): padding K to 128 only added DMA bytes and
    required GpSimd memsets that delayed the input DMA head.
  - Instances are packed in PAIRS (2 instances of the same image):
      layer0: lhsT [66, 128]  (cols 0-63 inst a, 64-127 inst b), one matmul
              computes both instances' 64 hidden channels.
      layer1: lhsT [128, 128] block-diagonal (w1a.T | w1b.T).
      layer2: lhsT [128, 32]  cols 0-1 used ([w2a;0] | [0;w2b]), 30 zero
              cols so the matmul initializes its full 32-partition group.
    Matmuls run in bf16 (fp32 PSUM accumulate); fp32r was measured 5x
    slower (no fast-weight-load, no background weight buffer).
  - The four layer2 matmuls of a quad (8 instances, one image) write ONE
    PSUM tile at partition offsets {0,32,64,96} via tile_position, and are
    emitted back-to-back at the end of each chunk: matmuls to distinct
    32-col groups of the PE array execute concurrently (per-subarray
    tiling), so the group costs ~1 matmul span instead of 4.
  - ReLU+bias PSUM->SBUF moves are the throughput-critical resource
    (~1.0us ACT / ~1.2us DVE per [128,1000] move, 1x perf mode since the
    source is fp32 PSUM). 9 moves/chunk split 5:4 between ACT and DVE.
  - HAM: the PE clock gate defaults to K=4/8 (1.2 GHz); ~3.4us of activity
    releases it to 2.4 GHz. A small warmup burst (8 matmuls on a zeroed
    tile) runs during the input-DMA head to pre-warm the gate; the old
    44-matmul burst serialized ~28us of PE time in front of real work.
"""

import numpy as np
from contextlib import ExitStack

N_IMG, C, H, W = 4, 64, 80, 200
NUM_INS = 8
M = N_IMG * NUM_INS          # 32 instances
N_CORES = 8
HPC = H // N_CORES           # 10 rows of H per core
SPI = HPC * W                # 2000 spatial positions per image slice
PAIRS = M // 2               # 16
CH = C + 2                   # 66 input channels incl. coords
FD = 1000                    # activation chunk
# matmul free-dim splits inside each 1000 chunk: PSUM banks hold 512 f32, and
# a matmul output must not cross a bank boundary -> split 512 + 488.
SPLITS = ((0, 512), (512, 488))

_W0N, _W1N, _W2N = CH * C, C * C, C
_B2_SHIFT = -2.19

_COMPILED = {}


def _build_program():
    import concourse.bacc as bacc
    import concourse.tile as tile
    from concourse import mybir

    dt = mybir.dt
    AF = mybir.ActivationFunctionType
    OP = mybir.AluOpType

    nc = bacc.Bacc("TRN2", target_bir_lowering=False, debug=False)

    # xs packs the 2 coordinate rows below the 64 x-channels so each image's
    # feats tile is filled by column-chunk DMAs only.
    xs_d = nc.dram_tensor("xs", [N_IMG, CH, SPI], dt.bfloat16, kind="ExternalInput").ap()
    l0_d = nc.dram_tensor("l0t", [CH, PAIRS * 128], dt.bfloat16, kind="ExternalInput").ap()
    l1_d = nc.dram_tensor("l1t", [128, PAIRS * 128], dt.bfloat16, kind="ExternalInput").ap()
    # layer2 runs in bf16: fp32r matmuls require dst start_partition == 0,
    # which the quad partition-packing (offsets 32/64/96) violates.
    l2_d = nc.dram_tensor("l2t", [128, PAIRS * 32], dt.bfloat16, kind="ExternalInput").ap()
    b0_d = nc.dram_tensor("b0t", [128, PAIRS], dt.float32, kind="ExternalInput").ap()
    b1_d = nc.dram_tensor("b1t", [128, PAIRS], dt.float32, kind="ExternalInput").ap()
    b2_d = nc.dram_tensor("b2q", [128, 4], dt.float32, kind="ExternalInput").ap()
    # out[q, j, r, :] = instance 8q + 2j + r, i.e. plain instance-major order
    out_d = nc.dram_tensor("out", [4, 4, 2, SPI], dt.float32, kind="ExternalOutput").ap()

    f32 = dt.float32
    bf16 = dt.bfloat16

    with tile.TileContext(nc) as tc, ExitStack() as ctx:
        cpool = ctx.enter_context(tc.tile_pool(name="const", bufs=1))
        hpool = ctx.enter_context(tc.tile_pool(name="work", bufs=4))
        pspool = ctx.enter_context(tc.tile_pool(name="ps", bufs=3, space="PSUM"))
        psqpool = ctx.enter_context(tc.tile_pool(name="psq", bufs=1, space="PSUM"))

        # ---- PE/ACT warmup ----
        # wsrc memset is tiny; the dummy Relu makes the ACT table-set DMA
        # issue at t~0 instead of queueing behind input DMAs, and the small
        # matmul burst warms the HAM clock gate during the input-DMA head.
        wsrc = cpool.tile([128, 640], bf16, tag="wsrc", name="wsrc")
        nc.gpsimd.memset(wsrc[:], 0.0)
        wact = cpool.tile([128, 8], f32, tag="wact", name="wact")
        nc.scalar.activation(wact[:], wsrc[:, 0:8], AF.Relu, bias=0.0)
        wps = pspool.tile([128, FD], f32, tag="ps", name="wps")
        for _ in range(8):
            nc.tensor.matmul(
                wps[:, 0:512], wsrc[:, 0:128], wsrc[:, 128:640],
                start=True, stop=True,
            )

        # ---- resident tiles + loads ----
        # Input DMA issues cost ~0.6us each on the issuing queue; alternate
        # between the two HW-DGE queues (sync + scalar) so image 0's data
        # lands as early as possible. Image-0 tensors (and the biases the
        # first ReLU needs) go first.
        b0s = cpool.tile([128, PAIRS], f32, tag="b0s", name="b0s")
        nc.sync.dma_start(b0s[:], b0_d[:])
        b1s = cpool.tile([128, PAIRS], f32, tag="b1s", name="b1s")
        nc.scalar.dma_start(b1s[:], b1_d[:])
        b2s = cpool.tile([128, 4], f32, tag="b2s", name="b2s")
        nc.sync.dma_start(b2s[:], b2_d[:])

        fe = []
        l0s = []
        l1s = []
        qs = [nc.sync, nc.scalar]
        qi = 0

        def q():
            nonlocal qi
            qi += 1
            return qs[qi % 2]

        for n in range(N_IMG):
            g0 = cpool.tile([CH, 4 * 128], bf16, tag=f"l0g{n}", name=f"l0g{n}")
            q().dma_start(g0[:], l0_d[:, n * 512 : (n + 1) * 512])
            l0s.append(g0)
            t = cpool.tile([CH, SPI], bf16, tag=f"fe{n}", name=f"fe{n}")
            # column-chunk split: 4 DMAs spread over engines, and the first
            # chunk's worth of data arrives ~4x sooner than one big DMA
            for a, b in ((0, 500), (500, 1000), (1000, 1500), (1500, 2000)):
                q().dma_start(t[:, a:b], xs_d[n, :, a:b])
            fe.append(t)
            g1 = cpool.tile([128, 4 * 128], bf16, tag=f"l1g{n}", name=f"l1g{n}")
            q().dma_start(g1[:], l1_d[:, n * 512 : (n + 1) * 512])
            l1s.append(g1)
            if n == 0:
                l2s = cpool.tile([128, PAIRS * 32], bf16, tag="l2s", name="l2s")
                q().dma_start(l2s[:], l2_d[:])

        # ---- main loop: quads of pairs (8 instances of one image) ----
        for qd in range(4):
            img = qd
            for hh in range(SPI // FD):
                base = hh * FD
                psq = psqpool.tile([128, FD], f32, tag="psq", name="psq")
                h2s = []
                for j in range(4):
                    p = 4 * qd + j
                    w0 = l0s[img][:, j * 128 : (j + 1) * 128]
                    w1 = l1s[img][:, j * 128 : (j + 1) * 128]
                    ps0 = pspool.tile([128, FD], f32, tag="ps", name="ps0")
                    for off, sz in SPLITS:
                        nc.tensor.matmul(
                            ps0[:, off : off + sz],
                            w0,
                            fe[img][:, base + off : base + off + sz],
                            start=True,
                            stop=True,
                        )
                    h1 = hpool.tile([128, FD], bf16, tag="h1", name="h1")
                    if j % 2 == 0:
                        nc.scalar.activation(
                            h1[:], ps0[:], AF.Relu, bias=b0s[:, p : p + 1]
                        )
                    else:
                        nc.vector.tensor_scalar(
                            h1[:], ps0[:], b0s[:, p : p + 1], 0.0, OP.add, OP.max
                        )
                    ps1 = pspool.tile([128, FD], f32, tag="ps", name="ps1")
                    for off, sz in SPLITS:
                        nc.tensor.matmul(
                            ps1[:, off : off + sz],
                            w1,
                            h1[:, off : off + sz],
                            start=True,
                            stop=True,
                        )
                    h2 = hpool.tile([128, FD], bf16, tag="h2", name="h2", bufs=6)
                    if j % 2 == 0:
                        nc.vector.tensor_scalar(
                            h2[:], ps1[:], b1s[:, p : p + 1], 0.0, OP.add, OP.max
                        )
                    else:
                        nc.scalar.activation(
                            h2[:], ps1[:], AF.Relu, bias=b1s[:, p : p + 1]
                        )
                    h2s.append(h2)
                # layer2 for all 4 pairs, grouped back-to-back: the four
                # matmuls per split target distinct 32-col groups of the PE
                # array (tile_position) and execute concurrently.
                for off, sz in SPLITS:
                    for j in range(4):
                        p = 4 * qd + j
                        w2 = l2s[:, 32 * p : 32 * p + 32]
                        nc.tensor.matmul(
                            psq[32 * j : 32 * j + 32, off : off + sz],
                            w2,
                            h2s[j][:, off : off + sz],
                            start=True,
                            stop=True,
                            tile_position=(0, 32 * j),
                        )
                # fused bias+move for the (quad, half) on ACT (it is the
                # faster mover; this keeps ACT:DVE at 5:4 moves per chunk).
                # Dedicated oq tiles (no pool reuse) let the output DMAs
                # drain lazily without write-after-read hazards.
                oq = cpool.tile([128, FD], f32, tag=f"oq{qd}_{hh}", name=f"oq{qd}_{hh}")
                nc.scalar.activation(
                    oq[:], psq[:], AF.Identity, bias=b2s[:, qd : qd + 1]
                )
                # contiguous 2-row reads (strided-partition reads miss deps
                # in the tile tracker and raced the move op). gpsimd is the
                # otherwise-idle issue queue; late in the run sync is idle
                # too, so alternate to halve the issue serialization.
                for j in range(4):
                    eng = nc.gpsimd if (2 * qd + hh) < 4 or j % 2 == 0 else nc.sync
                    eng.dma_start(
                        out_d[qd, j, :, base : base + FD], oq[32 * j : 32 * j + 2, :]
                    )

    nc.compile()
    _dedupe_ldweights(nc, mybir)
    return nc


def _dedupe_ldweights(nc, mybir):
    """Drop redundant PE LDWEIGHTS after compile.

    Tile emits one LDWEIGHTS per matmul; consecutive matmuls often share one
    stationary operand (split-column pairs, the warmup burst). The PE array
    is 16 independent 32x32 subarrays, so a narrow (<=32-col) load only
    clobbers its own col group: track the last-retained signature per col
    group and clear everything on a full-width load. Safe removal criteria:
    identical weights AP + tile_position as the last retained LDWEIGHTS for
    that col group, and no semaphore waits/updates on the dropped
    instruction, so synchronization is untouched.
    """
    dropped = 0
    for fn in nc.m.functions:
        for blk in fn.blocks:
            new = []
            last = {}  # col_grp -> sig
            for i in blk.instructions:
                if (
                    isinstance(i, mybir.InstLdweights)
                    and i.engine == mybir.EngineType.PE
                ):
                    tp = tuple(i.tile_position or ())
                    sig = (str(i.ins[0]), tp, i.perf_mode, i.is_transpose)
                    si = i.sync_info
                    clean = si is None or (not si.on_wait and not si.on_update)
                    try:
                        ncols = i.ins[0].free_size()
                    except Exception:
                        ncols = 128
                    key = tp[1] if len(tp) == 2 else 0
                    if ncols > 32:
                        # full/wide load touches multiple col groups
                        if clean and last.get("wide") == sig and len(last) == 1:
                            dropped += 1
                            continue
                        last = {"wide": sig}
                    else:
                        if clean and last.get(key) == sig:
                            dropped += 1
                            continue
                        last.pop("wide", None)
                        last[key] = sig
                new.append(i)
            if dropped:
                blk.instructions.clear()
                blk.instructions.extend(new)
    return dropped


def _pack_params(mask_head_params):
    """Split generated params and build the pair-packed device layouts."""
    p = np.ascontiguousarray(mask_head_params, dtype=np.float32)
    o0, o1, o2 = _W0N, _W0N + _W1N, _W0N + _W1N + _W2N
    w0 = p[:, :o0].reshape(M, C, CH)
    w1 = p[:, o0:o1].reshape(M, C, C)
    w2 = p[:, o1:o2].reshape(M, C)
    b0 = p[:, o2 : o2 + C]
    b1 = p[:, o2 + C : o2 + 2 * C]
    b2 = p[:, o2 + 2 * C :] + np.float32(_B2_SHIFT)

    import ml_dtypes as _mld

    # layer0 lhsT [66, 16*128]: rows 0-63 = x-channel weights, 64-65 = coord
    # weights; cols pair-major then (inst a | inst b).
    w0T = np.transpose(w0, (2, 0, 1))              # [66(cin), 32, 64]
    w0T = np.concatenate([w0T[2:], w0T[:2]], 0)    # x channels first, coords last
    l0t = np.ascontiguousarray(w0T.reshape(CH, M * C).astype(_mld.bfloat16))

    l1 = np.zeros((PAIRS, 128, 128), dtype=np.float32)
    l1[:, :C, :C] = np.transpose(w1[0::2], (0, 2, 1))
    l1[:, C:, C:] = np.transpose(w1[1::2], (0, 2, 1))
    l1t = np.ascontiguousarray(
        np.transpose(l1, (1, 0, 2)).reshape(128, PAIRS * 128).astype(_mld.bfloat16)
    )

    l2 = np.zeros((PAIRS, 128, 32), dtype=np.float32)
    l2[:, :C, 0] = w2[0::2]
    l2[:, C:, 1] = w2[1::2]
    l2t = np.ascontiguousarray(
        np.transpose(l2, (1, 0, 2)).reshape(128, PAIRS * 32).astype(_mld.bfloat16)
    )

    b0t = np.ascontiguousarray(np.concatenate([b0[0::2], b0[1::2]], 1).T)  # [128,16]
    b1t = np.ascontiguousarray(np.concatenate([b1[0::2], b1[1::2]], 1).T)
    # b2 packed to match the quad PSUM layout: rows 32j+r of col q hold
    # instance 8q + 2j + r.
    b2q = np.zeros((128, 4), dtype=np.float32)
    for qq in range(4):
        for j in range(4):
            b2q[32 * j, qq] = b2[8 * qq + 2 * j, 0]
            b2q[32 * j + 1, qq] = b2[8 * qq + 2 * j + 1, 0]
    return l0t, l1t, l2t, b0t, b1t, b2q


def _run(x, mask_head_params, trace=False, trace_kwargs=None):
    from concourse.bass_utils import run_bass_kernel_spmd

    if "nc" not in _COMPILED:
        _COMPILED["nc"] = _build_program()
    nc = _COMPILED["nc"]

    x = np.ascontiguousarray(x, dtype=np.float32)
    l0t, l1t, l2t, b0t, b1t, b2q = _pack_params(mask_head_params)

    xx = np.tile(np.arange(W, dtype=np.float32) / W, HPC)  # [2000]
    in_maps = []
    for k in range(N_CORES):
        h0 = k * HPC
        yy = np.repeat((h0 + np.arange(HPC, dtype=np.float32)) / W, W)
        coords = np.stack([xx, yy], 0)  # [2, 2000]
        import ml_dtypes as _mld

        xsl = x[:, :, h0 : h0 + HPC, :].reshape(N_IMG, C, SPI)
        xs = np.ascontiguousarray(
            np.concatenate(
                [xsl, np.broadcast_to(coords, (N_IMG, 2, SPI))], axis=1
            ).astype(_mld.bfloat16)
        )
        in_maps.append(
            {
                "xs": xs,
                "l0t": l0t,
                "l1t": l1t,
                "l2t": l2t,
                "b0t": b0t,
                "b1t": b1t,
                "b2q": b2q,
            }
        )

    res = run_bass_kernel_spmd(
        nc,
        in_maps,
        list(range(N_CORES)),
        trace=trace,
        **(trace_kwargs or {}),
    )

    out = np.empty((1, M, H, W), dtype=np.float32)
    for k in range(N_CORES):
        oc = res.results[k]["out"].reshape(M, HPC, W)
        out[0, :, k * HPC : (k + 1) * HPC, :] = oc
    return out, res


def kernel(x, mask_head_params, num_ins):
    n_ins = int(np.asarray(num_ins))
    assert n_ins == NUM_INS, f"kernel hardcoded for num_ins={NUM_INS}, got {n_ins}"
    out, _ = _run(x, mask_head_params)
    return out


# revision 4
# speedup vs baseline: 1.0620x; 1.0274x over previous
"""Trainium2 Bass kernel for CondLaneRNNHead-style dynamic mask head.

Computation (see reference): per-instance 3-layer 1x1-conv MLP over
per-image feature maps augmented with 2 coordinate channels.

  out[m] = w2[m] @ relu(w1[m] @ relu(w0[m] @ [coords; x[img(m)]] + b0[m]) + b1[m]) + b2[m]

Shapes: x [4, 64, 80, 200] f32, mask_head_params [32, 8513] f32, num_ins=8.
Output [1, 32, 80, 200] f32.

Sharding: spatial, along H. Core k processes rows [10k, 10k+10) of all 4
images for all 32 instances. This replicates only the (small) per-instance
params across cores; the big x tensor is read exactly once in aggregate.

Device kernel structure (per core):
  - feats[img] SBUF tile [66, 2000]: partitions 0-63 = x channels,
    partitions 64-65 = (xx/W, yy/W) coordinate rows. Layer0 matmuls run
    K=66 directly (no zero-pad): padding K to 128 only added DMA bytes and
    required GpSimd memsets that delayed the input DMA head.
  - Instances are packed in PAIRS (2 instances of the same image):
      layer0: lhsT [66, 128]  (cols 0-63 inst a, 64-127 inst b), one matmul
              computes both instances' 64 hidden channels.
      layer1: lhsT [128, 128] block-diagonal (w1a.T | w1b.T).
      layer2: lhsT [128, 32]  cols 0-1 used ([w2a;0] | [0;w2b]), 30 zero
              cols so the matmul initializes its full 32-partition group.
    Matmuls run in bf16 (fp32 PSUM accumulate); fp32r was measured 5x
    slower (no fast-weight-load, no background weight buffer).
  - The four layer2 matmuls of a quad (8 instances, one image) write ONE
    PSUM tile at partition offsets {0,32,64,96} via tile_position, and are
    emitted back-to-back at the end of each chunk: matmuls to distinct
    32-col groups of the PE array execute concurrently (per-subarray
    tiling), so the group costs ~1 matmul span instead of 4.
  - ReLU+bias PSUM->SBUF moves are the throughput-critical resource
    (~1.0us ACT / ~1.2us DVE per [128,1000] move, 1x perf mode since the
    source is fp32 PSUM). 9 moves/chunk split 5:4 between ACT and DVE.
  - HAM: the PE clock gate defaults to K=4/8 (1.2 GHz); ~3.4us of activity
    releases it to 2.4 GHz. A small warmup burst (8 matmuls on a zeroed
    tile) runs during the input-DMA head to pre-warm the gate; the old
    44-matmul burst serialized ~28us of PE time in front of real work.
"""

import numpy as np
from contextlib import ExitStack

N_IMG, C, H, W = 4, 64, 80, 200
NUM_INS = 8
M = N_IMG * NUM_INS          # 32 instances
N_CORES = 8
HPC = H // N_CORES           # 10 rows of H per core
SPI = HPC * W                # 2000 spatial positions per image slice
PAIRS = M // 2               # 16
CH = C + 2                   # 66 input channels incl. coords
FD = 1000                    # activation chunk
# matmul free-dim splits inside each 1000 chunk: PSUM banks hold 512 f32, and
# a matmul output must not cross a bank boundary -> split 512 + 488.
SPLITS = ((0, 512), (512, 488))

_W0N, _W1N, _W2N = CH * C, C * C, C
_B2_SHIFT = -2.19

_COMPILED = {}


def _build_program():
    import concourse.bacc as bacc
    import concourse.tile as tile
    from concourse import mybir

    dt = mybir.dt
    AF = mybir.ActivationFunctionType
    OP = mybir.AluOpType

    nc = bacc.Bacc("TRN2", target_bir_lowering=False, debug=False)

    # xs packs the 2 coordinate rows below the 64 x-channels so each image's
    # feats tile is filled by column-chunk DMAs only.
    xs_d = nc.dram_tensor("xs", [N_IMG, CH, SPI], dt.bfloat16, kind="ExternalInput").ap()
    l0_d = nc.dram_tensor("l0t", [CH, PAIRS * 128], dt.bfloat16, kind="ExternalInput").ap()
    l1_d = nc.dram_tensor("l1t", [128, PAIRS * 128], dt.bfloat16, kind="ExternalInput").ap()
    # layer2 runs in bf16: fp32r matmuls require dst start_partition == 0,
    # which the quad partition-packing (offsets 32/64/96) violates.
    l2_d = nc.dram_tensor("l2t", [128, PAIRS * 32], dt.bfloat16, kind="ExternalInput").ap()
    b0_d = nc.dram_tensor("b0t", [128, PAIRS], dt.float32, kind="ExternalInput").ap()
    b1_d = nc.dram_tensor("b1t", [128, PAIRS], dt.float32, kind="ExternalInput").ap()
    b2_d = nc.dram_tensor("b2q", [128, 4], dt.float32, kind="ExternalInput").ap()
    # out[q, j, r, :] = instance 8q + 2j + r, i.e. plain instance-major order
    out_d = nc.dram_tensor("out", [4, 4, 2, SPI], dt.float32, kind="ExternalOutput").ap()

    f32 = dt.float32
    bf16 = dt.bfloat16

    with tile.TileContext(nc) as tc, ExitStack() as ctx:
        cpool = ctx.enter_context(tc.tile_pool(name="const", bufs=1))
        hpool = ctx.enter_context(tc.tile_pool(name="work", bufs=4))
        pspool = ctx.enter_context(tc.tile_pool(name="ps", bufs=3, space="PSUM"))
        psqpool = ctx.enter_context(tc.tile_pool(name="psq", bufs=1, space="PSUM"))

        # ---- PE/ACT warmup ----
        # The dummy Relu makes the ACT table-set DMA issue at t~0 instead of
        # queueing behind input DMAs. The matmul burst warms the HAM clock
        # gate (default K=4/8 = 1.2 GHz; ~3.4us of activity releases it to
        # 2.4 GHz) during the input-DMA head; the burst is sized to bridge
        # until image-0 data lands (~11us) — the matmuls WAW-serialize on
        # wps (~0.6us each) so 8 of them cover ~5us of wall clock.
        wsrc = cpool.tile([128, 640], bf16, tag="wsrc", name="wsrc")
        nc.vector.memset(wsrc[:], 0.0)
        wact = cpool.tile([128, 8], f32, tag="wact", name="wact")
        nc.scalar.activation(wact[:], wsrc[:, 0:8], AF.Relu, bias=0.0)
        wps = pspool.tile([128, FD], f32, tag="ps", name="wps")
        for _ in range(8):
            nc.tensor.matmul(
                wps[:, 0:512], wsrc[:, 0:128], wsrc[:, 128:640],
                start=True, stop=True,
            )

        # ---- resident tiles + loads ----
        # DMA engines round-robin ALL queued transfers, so issuing
        # everything up front makes image 0's data finish only when the
        # whole 1.7MB input drains (~22us, measured). Instead: issue only
        # image 0+1 (and the small tensors) up front on the sync queue, and
        # gate images 2/3 behind tiny gpsimd reads of fe0/fe1 arrival so
        # their transfers cannot compete with the data the pipeline needs
        # first.
        b0s = cpool.tile([128, PAIRS], f32, tag="b0s", name="b0s")
        nc.sync.dma_start(b0s[:], b0_d[:])
        b1s = cpool.tile([128, PAIRS], f32, tag="b1s", name="b1s")
        nc.sync.dma_start(b1s[:], b1_d[:])
        b2s = cpool.tile([128, 4], f32, tag="b2s", name="b2s")
        nc.sync.dma_start(b2s[:], b2_d[:])

        fe = []
        l0s = []
        l1s = []
        for n in range(N_IMG):
            g0 = cpool.tile([CH, 4 * 128], bf16, tag=f"l0g{n}", name=f"l0g{n}")
            l0s.append(g0)
            fe.append(cpool.tile([CH, SPI], bf16, tag=f"fe{n}", name=f"fe{n}"))
            g1 = cpool.tile([128, 4 * 128], bf16, tag=f"l1g{n}", name=f"l1g{n}")
            l1s.append(g1)
        l2s = cpool.tile([128, PAIRS * 32], bf16, tag="l2s", name="l2s")

        def load_img(eng, n):
            eng.dma_start(l0s[n][:], l0_d[:, n * 512 : (n + 1) * 512])
            # column-chunk split: 4 DMAs spread over engines, and the first
            # chunk's worth of data arrives ~4x sooner than one big DMA
            for a, b in ((0, 500), (500, 1000), (1000, 1500), (1500, 2000)):
                eng.dma_start(fe[n][:, a:b], xs_d[n, :, a:b])
            eng.dma_start(l1s[n][:], l1_d[:, n * 512 : (n + 1) * 512])

        load_img(nc.sync, 0)
        nc.sync.dma_start(l2s[:], l2_d[:])
        load_img(nc.sync, 1)
        # fe0 fully landed -> issue image 2; fe1 landed -> image 3. The tiny
        # tensor_copy reads create the arrival dependency; gpsimd's in-order
        # queue then holds the dma issues behind it. (fe tiles are resident,
        # never rotated, so the extra reader is hazard-free.)
        dummy = cpool.tile([16, 8], bf16, tag="dummy", name="dummy")
        nc.gpsimd.tensor_copy(dummy[:], fe[0][0:16, 0:8])
        load_img(nc.gpsimd, 2)
        nc.gpsimd.tensor_copy(dummy[:], fe[1][0:16, 0:8])
        load_img(nc.gpsimd, 3)

        # ---- main loop: quads of pairs (8 instances of one image) ----
        for qd in range(4):
            img = qd
            for hh in range(SPI // FD):
                base = hh * FD
                psq = psqpool.tile([128, FD], f32, tag="psq", name="psq")
                h2s = []
                for j in range(4):
                    p = 4 * qd + j
                    w0 = l0s[img][:, j * 128 : (j + 1) * 128]
                    w1 = l1s[img][:, j * 128 : (j + 1) * 128]
                    ps0 = pspool.tile([128, FD], f32, tag="ps", name="ps0")
                    for off, sz in SPLITS:
                        nc.tensor.matmul(
                            ps0[:, off : off + sz],
                            w0,
                            fe[img][:, base + off : base + off + sz],
                            start=True,
                            stop=True,
                        )
                    h1 = hpool.tile([128, FD], bf16, tag="h1", name="h1")
                    if j % 2 == 0:
                        nc.scalar.activation(
                            h1[:], ps0[:], AF.Relu, bias=b0s[:, p : p + 1]
                        )
                    else:
                        nc.vector.tensor_scalar(
                            h1[:], ps0[:], b0s[:, p : p + 1], 0.0, OP.add, OP.max
                        )
                    ps1 = pspool.tile([128, FD], f32, tag="ps", name="ps1")
                    for off, sz in SPLITS:
                        nc.tensor.matmul(
                            ps1[:, off : off + sz],
                            w1,
                            h1[:, off : off + sz],
                            start=True,
                            stop=True,
                        )
                    h2 = hpool.tile([128, FD], bf16, tag="h2", name="h2", bufs=6)
                    if j % 2 == 0:
                        nc.vector.tensor_scalar(
                            h2[:], ps1[:], b1s[:, p : p + 1], 0.0, OP.add, OP.max
                        )
                    else:
                        nc.scalar.activation(
                            h2[:], ps1[:], AF.Relu, bias=b1s[:, p : p + 1]
                        )
                    h2s.append(h2)
                # layer2 for all 4 pairs, grouped back-to-back: the four
                # matmuls per split target distinct 32-col groups of the PE
                # array (tile_position) and execute concurrently.
                for off, sz in SPLITS:
                    for j in range(4):
                        p = 4 * qd + j
                        w2 = l2s[:, 32 * p : 32 * p + 32]
                        nc.tensor.matmul(
                            psq[32 * j : 32 * j + 32, off : off + sz],
                            w2,
                            h2s[j][:, off : off + sz],
                            start=True,
                            stop=True,
                            tile_position=(0, 32 * j),
                        )
                # fused bias+move for the (quad, half) on ACT (it is the
                # faster mover; this keeps ACT:DVE at 5:4 moves per chunk).
                # Dedicated oq tiles (no pool reuse) let the output DMAs
                # drain lazily without write-after-read hazards.
                oq = cpool.tile([128, FD], f32, tag=f"oq{qd}_{hh}", name=f"oq{qd}_{hh}")
                nc.scalar.activation(
                    oq[:], psq[:], AF.Identity, bias=b2s[:, qd : qd + 1]
                )
                # contiguous 2-row reads (strided-partition reads miss deps
                # in the tile tracker and raced the move op). sync is idle
                # once the input issues drain (~15us); split the last
                # chunk's issues with gpsimd so the tail doesn't serialize
                # behind 4 back-to-back 0.6us issue slots.
                last = qd == 3 and hh == 1
                for j in range(4):
                    eng = nc.gpsimd if (last and j % 2 == 1) else nc.sync
                    eng.dma_start(
                        out_d[qd, j, :, base : base + FD], oq[32 * j : 32 * j + 2, :]
                    )

    nc.compile()
    _dedupe_ldweights(nc, mybir)
    return nc


def _dedupe_ldweights(nc, mybir):
    """Drop redundant PE LDWEIGHTS after compile.

    Tile emits one LDWEIGHTS per matmul; consecutive matmuls often share one
    stationary operand (split-column pairs, the warmup burst). The PE array
    is 16 independent 32x32 subarrays, so a narrow (<=32-col) load only
    clobbers its own col group: track the last-retained signature per col
    group and clear everything on a full-width load. Safe removal criteria:
    identical weights AP + tile_position as the last retained LDWEIGHTS for
    that col group, and no semaphore waits/updates on the dropped
    instruction, so synchronization is untouched.
    """
    dropped = 0
    for fn in nc.m.functions:
        for blk in fn.blocks:
            new = []
            last = {}  # col_grp -> sig
            for i in blk.instructions:
                if (
                    isinstance(i, mybir.InstLdweights)
                    and i.engine == mybir.EngineType.PE
                ):
                    tp = tuple(i.tile_position or ())
                    sig = (str(i.ins[0]), tp, i.perf_mode, i.is_transpose)
                    si = i.sync_info
                    clean = si is None or (not si.on_wait and not si.on_update)
                    try:
                        ncols = i.ins[0].free_size()
                    except Exception:
                        ncols = 128
                    key = tp[1] if len(tp) == 2 else 0
                    if ncols > 32:
                        # full/wide load touches multiple col groups
                        if clean and last.get("wide") == sig and len(last) == 1:
                            dropped += 1
                            continue
                        last = {"wide": sig}
                    else:
                        if clean and last.get(key) == sig:
                            dropped += 1
                            continue
                        last.pop("wide", None)
                        last[key] = sig
                new.append(i)
            if dropped:
                blk.instructions.clear()
                blk.instructions.extend(new)
    return dropped


def _pack_params(mask_head_params):
    """Split generated params and build the pair-packed device layouts."""
    p = np.ascontiguousarray(mask_head_params, dtype=np.float32)
    o0, o1, o2 = _W0N, _W0N + _W1N, _W0N + _W1N + _W2N
    w0 = p[:, :o0].reshape(M, C, CH)
    w1 = p[:, o0:o1].reshape(M, C, C)
    w2 = p[:, o1:o2].reshape(M, C)
    b0 = p[:, o2 : o2 + C]
    b1 = p[:, o2 + C : o2 + 2 * C]
    b2 = p[:, o2 + 2 * C :] + np.float32(_B2_SHIFT)

    import ml_dtypes as _mld

    # layer0 lhsT [66, 16*128]: rows 0-63 = x-channel weights, 64-65 = coord
    # weights; cols pair-major then (inst a | inst b).
    w0T = np.transpose(w0, (2, 0, 1))              # [66(cin), 32, 64]
    w0T = np.concatenate([w0T[2:], w0T[:2]], 0)    # x channels first, coords last
    l0t = np.ascontiguousarray(w0T.reshape(CH, M * C).astype(_mld.bfloat16))

    l1 = np.zeros((PAIRS, 128, 128), dtype=np.float32)
    l1[:, :C, :C] = np.transpose(w1[0::2], (0, 2, 1))
    l1[:, C:, C:] = np.transpose(w1[1::2], (0, 2, 1))
    l1t = np.ascontiguousarray(
        np.transpose(l1, (1, 0, 2)).reshape(128, PAIRS * 128).astype(_mld.bfloat16)
    )

    l2 = np.zeros((PAIRS, 128, 32), dtype=np.float32)
    l2[:, :C, 0] = w2[0::2]
    l2[:, C:, 1] = w2[1::2]
    l2t = np.ascontiguousarray(
        np.transpose(l2, (1, 0, 2)).reshape(128, PAIRS * 32).astype(_mld.bfloat16)
    )

    b0t = np.ascontiguousarray(np.concatenate([b0[0::2], b0[1::2]], 1).T)  # [128,16]
    b1t = np.ascontiguousarray(np.concatenate([b1[0::2], b1[1::2]], 1).T)
    # b2 packed to match the quad PSUM layout: rows 32j+r of col q hold
    # instance 8q + 2j + r.
    b2q = np.zeros((128, 4), dtype=np.float32)
    for qq in range(4):
        for j in range(4):
            b2q[32 * j, qq] = b2[8 * qq + 2 * j, 0]
            b2q[32 * j + 1, qq] = b2[8 * qq + 2 * j + 1, 0]
    return l0t, l1t, l2t, b0t, b1t, b2q


def _run(x, mask_head_params, trace=False, trace_kwargs=None):
    from concourse.bass_utils import run_bass_kernel_spmd

    if "nc" not in _COMPILED:
        _COMPILED["nc"] = _build_program()
    nc = _COMPILED["nc"]

    x = np.ascontiguousarray(x, dtype=np.float32)
    l0t, l1t, l2t, b0t, b1t, b2q = _pack_params(mask_head_params)

    xx = np.tile(np.arange(W, dtype=np.float32) / W, HPC)  # [2000]
    in_maps = []
    for k in range(N_CORES):
        h0 = k * HPC
        yy = np.repeat((h0 + np.arange(HPC, dtype=np.float32)) / W, W)
        coords = np.stack([xx, yy], 0)  # [2, 2000]
        import ml_dtypes as _mld

        xsl = x[:, :, h0 : h0 + HPC, :].reshape(N_IMG, C, SPI)
        xs = np.ascontiguousarray(
            np.concatenate(
                [xsl, np.broadcast_to(coords, (N_IMG, 2, SPI))], axis=1
            ).astype(_mld.bfloat16)
        )
        in_maps.append(
            {
                "xs": xs,
                "l0t": l0t,
                "l1t": l1t,
                "l2t": l2t,
                "b0t": b0t,
                "b1t": b1t,
                "b2q": b2q,
            }
        )

    res = run_bass_kernel_spmd(
        nc,
        in_maps,
        list(range(N_CORES)),
        trace=trace,
        **(trace_kwargs or {}),
    )

    out = np.empty((1, M, H, W), dtype=np.float32)
    for k in range(N_CORES):
        oc = res.results[k]["out"].reshape(M, HPC, W)
        out[0, :, k * HPC : (k + 1) * HPC, :] = oc
    return out, res


def kernel(x, mask_head_params, num_ins):
    n_ins = int(np.asarray(num_ins))
    assert n_ins == NUM_INS, f"kernel hardcoded for num_ins={NUM_INS}, got {n_ins}"
    out, _ = _run(x, mask_head_params)
    return out


# revision 6
# speedup vs baseline: 1.0809x; 1.0179x over previous
"""Trainium2 Bass kernel for CondLaneRNNHead-style dynamic mask head.

Computation (see reference): per-instance 3-layer 1x1-conv MLP over
per-image feature maps augmented with 2 coordinate channels.

  out[m] = w2[m] @ relu(w1[m] @ relu(w0[m] @ [coords; x[img(m)]] + b0[m]) + b1[m]) + b2[m]

Shapes: x [4, 64, 80, 200] f32, mask_head_params [32, 8513] f32, num_ins=8.
Output [1, 32, 80, 200] f32.

Sharding: spatial, along H. Core k processes rows [10k, 10k+10) of all 4
images for all 32 instances. This replicates only the (small) per-instance
params across cores; the big x tensor is read exactly once in aggregate.

Device kernel structure (per core):
  - feats[img] SBUF tile [66, 2000]: partitions 0-63 = x channels,
    partitions 64-65 = (xx/W, yy/W) coordinate rows. Layer0 matmuls run
    K=66 directly (no zero-pad): padding K to 128 only added DMA bytes and
    required GpSimd memsets that delayed the input DMA head.
  - Instances are packed in PAIRS (2 instances of the same image):
      layer0: lhsT [66, 128]  (cols 0-63 inst a, 64-127 inst b), one matmul
              computes both instances' 64 hidden channels.
      layer1: lhsT [128, 128] block-diagonal (w1a.T | w1b.T).
      layer2: lhsT [128, 32]  cols 0-1 used ([w2a;0] | [0;w2b]), 30 zero
              cols so the matmul initializes its full 32-partition group.
    Matmuls run in bf16 (fp32 PSUM accumulate); fp32r was measured 5x
    slower (no fast-weight-load, no background weight buffer).
  - The four layer2 matmuls of a quad (8 instances, one image) write ONE
    PSUM tile at partition offsets {0,32,64,96} via tile_position, and are
    emitted back-to-back at the end of each chunk: matmuls to distinct
    32-col groups of the PE array execute concurrently (per-subarray
    tiling), so the group costs ~1 matmul span instead of 4.
  - ReLU+bias PSUM->SBUF moves are the throughput-critical resource
    (~1.0us ACT / ~1.2us DVE per [128,1000] move, 1x perf mode since the
    source is fp32 PSUM). 9 moves/chunk split 5:4 between ACT and DVE.
  - HAM: the PE clock gate defaults to K=4/8 (1.2 GHz); ~3.4us of activity
    releases it to 2.4 GHz. A small warmup burst (8 matmuls on a zeroed
    tile) runs during the input-DMA head to pre-warm the gate; the old
    44-matmul burst serialized ~28us of PE time in front of real work.
"""

import numpy as np
from contextlib import ExitStack

N_IMG, C, H, W = 4, 64, 80, 200
NUM_INS = 8
M = N_IMG * NUM_INS          # 32 instances
N_CORES = 8
HPC = H // N_CORES           # 10 rows of H per core
SPI = HPC * W                # 2000 spatial positions per image slice
PAIRS = M // 2               # 16
CH = C + 2                   # 66 input channels incl. coords
FD = 1000                    # activation chunk
# matmul free-dim splits inside each 1000 chunk: PSUM banks hold 512 f32, and
# a matmul output must not cross a bank boundary -> split 512 + 488.
SPLITS = ((0, 512), (512, 488))

_W0N, _W1N, _W2N = CH * C, C * C, C
_B2_SHIFT = -2.19

_COMPILED = {}


def _build_program():
    import concourse.bacc as bacc
    import concourse.tile as tile
    from concourse import mybir

    dt = mybir.dt
    AF = mybir.ActivationFunctionType
    OP = mybir.AluOpType

    nc = bacc.Bacc("TRN2", target_bir_lowering=False, debug=False)

    # xs packs the 2 coordinate rows below the 64 x-channels so each image's
    # feats tile is filled by column-chunk DMAs only.
    xs_d = nc.dram_tensor("xs", [N_IMG, CH, SPI], dt.bfloat16, kind="ExternalInput").ap()
    l0_d = nc.dram_tensor("l0t", [CH, PAIRS * 128], dt.bfloat16, kind="ExternalInput").ap()
    l1_d = nc.dram_tensor("l1t", [128, PAIRS * 128], dt.bfloat16, kind="ExternalInput").ap()
    # layer2 runs in bf16: fp32r matmuls require dst start_partition == 0,
    # which the quad partition-packing (offsets 32/64/96) violates.
    l2_d = nc.dram_tensor("l2t", [128, PAIRS * 32], dt.bfloat16, kind="ExternalInput").ap()
    b0_d = nc.dram_tensor("b0t", [128, PAIRS], dt.float32, kind="ExternalInput").ap()
    b1_d = nc.dram_tensor("b1t", [128, PAIRS], dt.float32, kind="ExternalInput").ap()
    b2_d = nc.dram_tensor("b2q", [128, 4], dt.float32, kind="ExternalInput").ap()
    # out[q, j, r, :] = instance 8q + 2j + r, i.e. plain instance-major order
    out_d = nc.dram_tensor("out", [4, 4, 2, SPI], dt.float32, kind="ExternalOutput").ap()

    f32 = dt.float32
    bf16 = dt.bfloat16

    with tile.TileContext(nc) as tc, ExitStack() as ctx:
        cpool = ctx.enter_context(tc.tile_pool(name="const", bufs=1))
        hpool = ctx.enter_context(tc.tile_pool(name="work", bufs=4))
        pspool = ctx.enter_context(tc.tile_pool(name="ps", bufs=3, space="PSUM"))
        psqpool = ctx.enter_context(tc.tile_pool(name="psq", bufs=1, space="PSUM"))

        # ---- PE/ACT warmup ----
        # The dummy Relu makes the ACT table-set DMA issue at t~0 instead of
        # queueing behind input DMAs. The matmul burst warms the HAM clock
        # gate (default K=4/8 = 1.2 GHz; ~3.4us of activity releases it to
        # 2.4 GHz) during the input-DMA head; the burst is sized to bridge
        # until image-0 data lands (~11us) — the matmuls WAW-serialize on
        # wps (~0.6us each) so 8 of them cover ~5us of wall clock.
        wsrc = cpool.tile([128, 640], bf16, tag="wsrc", name="wsrc")
        nc.vector.memset(wsrc[:], 0.0)
        wact = cpool.tile([128, 8], f32, tag="wact", name="wact")
        nc.scalar.activation(wact[:], wsrc[:, 0:8], AF.Relu, bias=0.0)
        wps = pspool.tile([128, FD], f32, tag="ps", name="wps")
        for _ in range(8):
            nc.tensor.matmul(
                wps[:, 0:512], wsrc[:, 0:128], wsrc[:, 128:640],
                start=True, stop=True,
            )

        # ---- resident tiles + loads ----
        # DMA engines round-robin ALL queued transfers, so issuing
        # everything up front makes image 0's data finish only when the
        # whole 1.7MB input drains (~22us, measured). Instead: issue only
        # image 0+1 (and the small tensors) up front on the sync queue, and
        # gate images 2/3 behind tiny gpsimd reads of fe0/fe1 arrival so
        # their transfers cannot compete with the data the pipeline needs
        # first.
        b0s = cpool.tile([128, PAIRS], f32, tag="b0s", name="b0s")
        b1s = cpool.tile([128, PAIRS], f32, tag="b1s", name="b1s")
        b2s = cpool.tile([128, 4], f32, tag="b2s", name="b2s")

        fe = []
        l0s = []
        l1s = []
        for n in range(N_IMG):
            g0 = cpool.tile([CH, 4 * 128], bf16, tag=f"l0g{n}", name=f"l0g{n}")
            l0s.append(g0)
            fe.append(cpool.tile([CH, SPI], bf16, tag=f"fe{n}", name=f"fe{n}"))
            g1 = cpool.tile([128, 4 * 128], bf16, tag=f"l1g{n}", name=f"l1g{n}")
            l1s.append(g1)
        l2s = cpool.tile([128, PAIRS * 32], bf16, tag="l2s", name="l2s")

        FECOLS = ((0, 500), (500, 1000), (1000, 1500), (1500, 2000))

        def load_img(eng, n):
            eng.dma_start(l0s[n][:], l0_d[:, n * 512 : (n + 1) * 512])
            # column-chunk split: 4 DMAs spread over engines, and the first
            # chunk's worth of data arrives ~4x sooner than one big DMA
            for a, b in FECOLS:
                eng.dma_start(fe[n][:, a:b], xs_d[n, :, a:b])
            eng.dma_start(l1s[n][:], l1_d[:, n * 512 : (n + 1) * 512])

        # First wave split across all three issue queues (each dma_start
        # costs ~0.6us of issue time on its queue) so image 0 is fully in
        # SBUF ~4us after the preamble, right as the warmup burst ends.
        nc.sync.dma_start(l0s[0][:], l0_d[:, 0:512])
        nc.sync.dma_start(fe[0][:, 0:500], xs_d[0, :, 0:500])
        nc.scalar.dma_start(fe[0][:, 500:1000], xs_d[0, :, 500:1000])
        nc.gpsimd.dma_start(fe[0][:, 1000:1500], xs_d[0, :, 1000:1500])
        nc.gpsimd.dma_start(fe[0][:, 1500:2000], xs_d[0, :, 1500:2000])
        nc.sync.dma_start(l1s[0][:], l1_d[:, 0:512])
        nc.scalar.dma_start(b0s[:], b0_d[:])
        nc.gpsimd.dma_start(b1s[:], b1_d[:])
        nc.gpsimd.dma_start(b2s[:], b2_d[:])
        # image 1 + l2 follow on sync (needed ~15us later)
        load_img(nc.sync, 1)
        nc.sync.dma_start(l2s[:], l2_d[:])
        # fe0 fully landed -> issue image 2; fe1 landed -> image 3. The tiny
        # tensor_copy reads create the arrival dependency; gpsimd's in-order
        # queue then holds the dma issues behind it. (fe tiles are resident,
        # never rotated, so the extra reader is hazard-free.)
        dummy = cpool.tile([16, 8], bf16, tag="dummy", name="dummy")
        nc.gpsimd.tensor_copy(dummy[:], fe[0][0:16, 0:8])
        load_img(nc.gpsimd, 2)
        nc.gpsimd.tensor_copy(dummy[:], fe[1][0:16, 0:8])
        load_img(nc.gpsimd, 3)

        # ---- main loop: quads of pairs (8 instances of one image) ----
        for qd in range(4):
            img = qd
            for hh in range(SPI // FD):
                base = hh * FD
                psq = psqpool.tile([128, FD], f32, tag="psq", name="psq")
                h2s = []
                for j in range(4):
                    p = 4 * qd + j
                    w0 = l0s[img][:, j * 128 : (j + 1) * 128]
                    w1 = l1s[img][:, j * 128 : (j + 1) * 128]
                    ps0 = pspool.tile([128, FD], f32, tag="ps", name="ps0")
                    for off, sz in SPLITS:
                        nc.tensor.matmul(
                            ps0[:, off : off + sz],
                            w0,
                            fe[img][:, base + off : base + off + sz],
                            start=True,
                            stop=True,
                        )
                    h1 = hpool.tile([128, FD], bf16, tag="h1", name="h1")
                    if j % 2 == 0:
                        nc.scalar.activation(
                            h1[:], ps0[:], AF.Relu, bias=b0s[:, p : p + 1]
                        )
                    else:
                        nc.vector.tensor_scalar(
                            h1[:], ps0[:], b0s[:, p : p + 1], 0.0, OP.add, OP.max
                        )
                    ps1 = pspool.tile([128, FD], f32, tag="ps", name="ps1")
                    for off, sz in SPLITS:
                        nc.tensor.matmul(
                            ps1[:, off : off + sz],
                            w1,
                            h1[:, off : off + sz],
                            start=True,
                            stop=True,
                        )
                    h2 = hpool.tile([128, FD], bf16, tag="h2", name="h2", bufs=6)
                    if j % 2 == 0:
                        nc.vector.tensor_scalar(
                            h2[:], ps1[:], b1s[:, p : p + 1], 0.0, OP.add, OP.max
                        )
                    else:
                        nc.scalar.activation(
                            h2[:], ps1[:], AF.Relu, bias=b1s[:, p : p + 1]
                        )
                    h2s.append(h2)
                # layer2 for all 4 pairs, grouped back-to-back: the four
                # matmuls per split target distinct 32-col groups of the PE
                # array (tile_position) and execute concurrently.
                for off, sz in SPLITS:
                    for j in range(4):
                        p = 4 * qd + j
                        w2 = l2s[:, 32 * p : 32 * p + 32]
                        nc.tensor.matmul(
                            psq[32 * j : 32 * j + 32, off : off + sz],
                            w2,
                            h2s[j][:, off : off + sz],
                            start=True,
                            stop=True,
                            tile_position=(0, 32 * j),
                        )
                # fused bias+move for the (quad, half) on ACT (it is the
                # faster mover; this keeps ACT:DVE at 5:4 moves per chunk).
                # Dedicated oq tiles (no pool reuse) let the output DMAs
                # drain lazily without write-after-read hazards.
                oq = cpool.tile([128, FD], f32, tag=f"oq{qd}_{hh}", name=f"oq{qd}_{hh}")
                nc.scalar.activation(
                    oq[:], psq[:], AF.Identity, bias=b2s[:, qd : qd + 1]
                )
                # contiguous 2-row reads (strided-partition reads miss deps
                # in the tile tracker and raced the move op). All on sync:
                # it is idle once the input issues drain, and keeping
                # gpsimd's software-DGE queues untouched late in the run
                # avoids a ~2us queue-drain stall in the epilogue.
                for j in range(4):
                    nc.sync.dma_start(
                        out_d[qd, j, :, base : base + FD], oq[32 * j : 32 * j + 2, :]
                    )

    nc.compile()
    _dedupe_ldweights(nc, mybir)
    return nc


def _dedupe_ldweights(nc, mybir):
    """Drop redundant PE LDWEIGHTS after compile.

    Tile emits one LDWEIGHTS per matmul; consecutive matmuls often share one
    stationary operand (split-column pairs, the warmup burst). The PE array
    is 16 independent 32x32 subarrays, so a narrow (<=32-col) load only
    clobbers its own col group: track the last-retained signature per col
    group and clear everything on a full-width load. Safe removal criteria:
    identical weights AP + tile_position as the last retained LDWEIGHTS for
    that col group, and no semaphore waits/updates on the dropped
    instruction, so synchronization is untouched.
    """
    dropped = 0
    for fn in nc.m.functions:
        for blk in fn.blocks:
            new = []
            last = {}  # col_grp -> sig
            for i in blk.instructions:
                if (
                    isinstance(i, mybir.InstLdweights)
                    and i.engine == mybir.EngineType.PE
                ):
                    tp = tuple(i.tile_position or ())
                    sig = (str(i.ins[0]), tp, i.perf_mode, i.is_transpose)
                    si = i.sync_info
                    clean = si is None or (not si.on_wait and not si.on_update)
                    try:
                        ncols = i.ins[0].free_size()
                    except Exception:
                        ncols = 128
                    key = tp[1] if len(tp) == 2 else 0
                    if ncols > 32:
                        # full/wide load touches multiple col groups
                        if clean and last.get("wide") == sig and len(last) == 1:
                            dropped += 1
                            continue
                        last = {"wide": sig}
                    else:
                        if clean and last.get(key) == sig:
                            dropped += 1
                            continue
                        last.pop("wide", None)
                        last[key] = sig
                new.append(i)
            if dropped:
                blk.instructions.clear()
                blk.instructions.extend(new)
    return dropped


def _pack_params(mask_head_params):
    """Split generated params and build the pair-packed device layouts."""
    p = np.ascontiguousarray(mask_head_params, dtype=np.float32)
    o0, o1, o2 = _W0N, _W0N + _W1N, _W0N + _W1N + _W2N
    w0 = p[:, :o0].reshape(M, C, CH)
    w1 = p[:, o0:o1].reshape(M, C, C)
    w2 = p[:, o1:o2].reshape(M, C)
    b0 = p[:, o2 : o2 + C]
    b1 = p[:, o2 + C : o2 + 2 * C]
    b2 = p[:, o2 + 2 * C :] + np.float32(_B2_SHIFT)

    import ml_dtypes as _mld

    # layer0 lhsT [66, 16*128]: rows 0-63 = x-channel weights, 64-65 = coord
    # weights; cols pair-major then (inst a | inst b).
    w0T = np.transpose(w0, (2, 0, 1))              # [66(cin), 32, 64]
    w0T = np.concatenate([w0T[2:], w0T[:2]], 0)    # x channels first, coords last
    l0t = np.ascontiguousarray(w0T.reshape(CH, M * C).astype(_mld.bfloat16))

    l1 = np.zeros((PAIRS, 128, 128), dtype=np.float32)
    l1[:, :C, :C] = np.transpose(w1[0::2], (0, 2, 1))
    l1[:, C:, C:] = np.transpose(w1[1::2], (0, 2, 1))
    l1t = np.ascontiguousarray(
        np.transpose(l1, (1, 0, 2)).reshape(128, PAIRS * 128).astype(_mld.bfloat16)
    )

    l2 = np.zeros((PAIRS, 128, 32), dtype=np.float32)
    l2[:, :C, 0] = w2[0::2]
    l2[:, C:, 1] = w2[1::2]
    l2t = np.ascontiguousarray(
        np.transpose(l2, (1, 0, 2)).reshape(128, PAIRS * 32).astype(_mld.bfloat16)
    )

    b0t = np.ascontiguousarray(np.concatenate([b0[0::2], b0[1::2]], 1).T)  # [128,16]
    b1t = np.ascontiguousarray(np.concatenate([b1[0::2], b1[1::2]], 1).T)
    # b2 packed to match the quad PSUM layout: rows 32j+r of col q hold
    # instance 8q + 2j + r.
    b2q = np.zeros((128, 4), dtype=np.float32)
    for qq in range(4):
        for j in range(4):
            b2q[32 * j, qq] = b2[8 * qq + 2 * j, 0]
            b2q[32 * j + 1, qq] = b2[8 * qq + 2 * j + 1, 0]
    return l0t, l1t, l2t, b0t, b1t, b2q


def _run(x, mask_head_params, trace=False, trace_kwargs=None):
    from concourse.bass_utils import run_bass_kernel_spmd

    if "nc" not in _COMPILED:
        _COMPILED["nc"] = _build_program()
    nc = _COMPILED["nc"]

    x = np.ascontiguousarray(x, dtype=np.float32)
    l0t, l1t, l2t, b0t, b1t, b2q = _pack_params(mask_head_params)

    xx = np.tile(np.arange(W, dtype=np.float32) / W, HPC)  # [2000]
    in_maps = []
    for k in range(N_CORES):
        h0 = k * HPC
        yy = np.repeat((h0 + np.arange(HPC, dtype=np.float32)) / W, W)
        coords = np.stack([xx, yy], 0)  # [2, 2000]
        import ml_dtypes as _mld

        xsl = x[:, :, h0 : h0 + HPC, :].reshape(N_IMG, C, SPI)
        xs = np.ascontiguousarray(
            np.concatenate(
                [xsl, np.broadcast_to(coords, (N_IMG, 2, SPI))], axis=1
            ).astype(_mld.bfloat16)
        )
        in_maps.append(
            {
                "xs": xs,
                "l0t": l0t,
                "l1t": l1t,
                "l2t": l2t,
                "b0t": b0t,
                "b1t": b1t,
                "b2q": b2q,
            }
        )

    res = run_bass_kernel_spmd(
        nc,
        in_maps,
        list(range(N_CORES)),
        trace=trace,
        **(trace_kwargs or {}),
    )

    out = np.empty((1, M, H, W), dtype=np.float32)
    for k in range(N_CORES):
        oc = res.results[k]["out"].reshape(M, HPC, W)
        out[0, :, k * HPC : (k + 1) * HPC, :] = oc
    return out, res


def kernel(x, mask_head_params, num_ins):
    n_ins = int(np.asarray(num_ins))
    assert n_ins == NUM_INS, f"kernel hardcoded for num_ins={NUM_INS}, got {n_ins}"
    out, _ = _run(x, mask_head_params)
    return out


# revision 9
# speedup vs baseline: 1.1114x; 1.0282x over previous
"""Trainium2 Bass kernel for CondLaneRNNHead-style dynamic mask head.

Computation (see reference): per-instance 3-layer 1x1-conv MLP over
per-image feature maps augmented with 2 coordinate channels.

  out[m] = w2[m] @ relu(w1[m] @ relu(w0[m] @ [coords; x[img(m)]] + b0[m]) + b1[m]) + b2[m]

Shapes: x [4, 64, 80, 200] f32, mask_head_params [32, 8513] f32, num_ins=8.
Output [1, 32, 80, 200] f32.

Sharding: spatial, along H. Core k processes rows [10k, 10k+10) of all 4
images for all 32 instances. This replicates only the (small) per-instance
params across cores; the big x tensor is read exactly once in aggregate.

Device kernel structure (per core):
  - feats[img] SBUF tile [66, 2000]: partitions 0-63 = x channels,
    partitions 64-65 = (xx/W, yy/W) coordinate rows. Layer0 matmuls run
    K=66 directly (no zero-pad): padding K to 128 only added DMA bytes and
    required GpSimd memsets that delayed the input DMA head.
  - Instances are packed in PAIRS (2 instances of the same image):
      layer0: lhsT [66, 128]  (cols 0-63 inst a, 64-127 inst b), one matmul
              computes both instances' 64 hidden channels.
      layer1: lhsT [128, 128] block-diagonal (w1a.T | w1b.T).
      layer2: lhsT [128, 32]  cols 0-1 used ([w2a;0] | [0;w2b]), 30 zero
              cols so the matmul initializes its full 32-partition group.
    Matmuls run in bf16 (fp32 PSUM accumulate); fp32r was measured 5x
    slower (no fast-weight-load, no background weight buffer).
  - The four layer2 matmuls of a quad (8 instances, one image) write ONE
    PSUM tile at partition offsets {0,32,64,96} via tile_position, and are
    emitted back-to-back at the end of each chunk: matmuls to distinct
    32-col groups of the PE array execute concurrently (per-subarray
    tiling), so the group costs ~1 matmul span instead of 4.
  - ReLU+bias PSUM->SBUF moves are the throughput-critical resource
    (~1.0us ACT / ~1.2us DVE per [128,1000] move, 1x perf mode since the
    source is fp32 PSUM). 9 moves/chunk split 5:4 between ACT and DVE.
  - HAM: the PE clock gate defaults to K=4/8 (1.2 GHz); ~3.4us of activity
    releases it to 2.4 GHz. A small warmup burst (8 matmuls on a zeroed
    tile) runs during the input-DMA head to pre-warm the gate; the old
    44-matmul burst serialized ~28us of PE time in front of real work.
"""

import numpy as np
from contextlib import ExitStack

N_IMG, C, H, W = 4, 64, 80, 200
NUM_INS = 8
M = N_IMG * NUM_INS          # 32 instances
N_CORES = 8
HPC = H // N_CORES           # 10 rows of H per core
SPI = HPC * W                # 2000 spatial positions per image slice
PAIRS = M // 2               # 16
CH = C + 2                   # 66 input channels incl. coords
FD = 1000                    # activation chunk
# matmul free-dim splits inside each 1000 chunk: PSUM banks hold 512 f32, and
# a matmul output must not cross a bank boundary -> split 512 + 488.
SPLITS = ((0, 512), (512, 488))

_W0N, _W1N, _W2N = CH * C, C * C, C
_B2_SHIFT = -2.19

_COMPILED = {}


def _build_program():
    import concourse.bacc as bacc
    import concourse.tile as tile
    from concourse import mybir

    dt = mybir.dt
    AF = mybir.ActivationFunctionType
    OP = mybir.AluOpType

    nc = bacc.Bacc("TRN2", target_bir_lowering=False, debug=False)

    # xs packs the 2 coordinate rows below the 64 x-channels so each image's
    # feats tile is filled by column-chunk DMAs only.
    xs_d = nc.dram_tensor("xs", [N_IMG, CH, SPI], dt.bfloat16, kind="ExternalInput").ap()
    l0_d = nc.dram_tensor("l0t", [CH, PAIRS * 128], dt.bfloat16, kind="ExternalInput").ap()
    l1_d = nc.dram_tensor("l1t", [128, PAIRS * 128], dt.bfloat16, kind="ExternalInput").ap()
    # layer2 runs in bf16: fp32r matmuls require dst start_partition == 0,
    # which the quad partition-packing (offsets 32/64/96) violates.
    l2_d = nc.dram_tensor("l2t", [128, PAIRS * 32], dt.bfloat16, kind="ExternalInput").ap()
    b0_d = nc.dram_tensor("b0t", [128, PAIRS], dt.float32, kind="ExternalInput").ap()
    b1_d = nc.dram_tensor("b1t", [128, PAIRS], dt.float32, kind="ExternalInput").ap()
    b2_d = nc.dram_tensor("b2q", [128, 4], dt.float32, kind="ExternalInput").ap()
    # out[q, j, r, :] = instance 8q + 2j + r, i.e. plain instance-major order
    out_d = nc.dram_tensor("out", [4, 4, 2, SPI], dt.float32, kind="ExternalOutput").ap()

    f32 = dt.float32
    bf16 = dt.bfloat16

    with tile.TileContext(nc) as tc, ExitStack() as ctx:
        cpool = ctx.enter_context(tc.tile_pool(name="const", bufs=1))
        hpool = ctx.enter_context(tc.tile_pool(name="work", bufs=4))
        pspool = ctx.enter_context(tc.tile_pool(name="ps", bufs=3, space="PSUM"))
        psqpool = ctx.enter_context(tc.tile_pool(name="psq", bufs=1, space="PSUM"))

        # ---- PE/ACT warmup ----
        # The dummy Relu makes the ACT table-set DMA issue at t~0 instead of
        # queueing behind input DMAs. The matmul burst warms the HAM clock
        # gate (default K=4/8 = 1.2 GHz; ~3.4us of activity releases it to
        # 2.4 GHz) during the input-DMA head; the burst is sized to bridge
        # until image-0 data lands (~11us) — the matmuls WAW-serialize on
        # wps (~0.6us each) so 8 of them cover ~5us of wall clock.
        wsrc = cpool.tile([128, 640], bf16, tag="wsrc", name="wsrc")
        nc.vector.memset(wsrc[:], 0.0)
        wact = cpool.tile([128, 8], f32, tag="wact", name="wact")
        nc.scalar.activation(wact[:], wsrc[:, 0:8], AF.Relu, bias=0.0)
        wps = pspool.tile([128, FD], f32, tag="ps", name="wps")
        for _ in range(10):
            nc.tensor.matmul(
                wps[:, 0:512], wsrc[:, 0:128], wsrc[:, 128:640],
                start=True, stop=True,
            )

        # ---- resident tiles + loads ----
        # DMA engines round-robin ALL queued transfers, so issuing
        # everything up front makes image 0's data finish only when the
        # whole 1.7MB input drains (~22us, measured). Instead: issue only
        # image 0+1 (and the small tensors) up front on the sync queue, and
        # gate images 2/3 behind tiny gpsimd reads of fe0/fe1 arrival so
        # their transfers cannot compete with the data the pipeline needs
        # first.
        b0s = cpool.tile([128, PAIRS], f32, tag="b0s", name="b0s")
        b1s = cpool.tile([128, PAIRS], f32, tag="b1s", name="b1s")
        b2s = cpool.tile([128, 4], f32, tag="b2s", name="b2s")

        fe = []
        l0s = []
        l1s = []
        for n in range(N_IMG):
            g0 = cpool.tile([CH, 4 * 128], bf16, tag=f"l0g{n}", name=f"l0g{n}")
            l0s.append(g0)
            fe.append(cpool.tile([CH, SPI], bf16, tag=f"fe{n}", name=f"fe{n}"))
            g1 = cpool.tile([128, 4 * 128], bf16, tag=f"l1g{n}", name=f"l1g{n}")
            l1s.append(g1)
        l2s = cpool.tile([128, PAIRS * 32], bf16, tag="l2s", name="l2s")

        FECOLS = ((0, 500), (500, 1000), (1000, 1500), (1500, 2000))

        def load_img(eng, n):
            eng.dma_start(l0s[n][:], l0_d[:, n * 512 : (n + 1) * 512])
            # column-chunk split: 4 DMAs spread over engines, and the first
            # chunk's worth of data arrives ~4x sooner than one big DMA
            for a, b in FECOLS:
                eng.dma_start(fe[n][:, a:b], xs_d[n, :, a:b])
            eng.dma_start(l1s[n][:], l1_d[:, n * 512 : (n + 1) * 512])

        # First wave split across all three issue queues (each dma_start
        # costs ~0.6us of issue time on its queue) so image 0 is fully in
        # SBUF ~5us after the preamble, right as the warmup burst ends.
        # DMA engines round-robin ALL queued transfers, so nothing else may
        # be in flight until image 0 lands: every later image is gated
        # behind the previous image's feature arrival.
        nc.sync.dma_start(l0s[0][:], l0_d[:, 0:512])
        nc.sync.dma_start(fe[0][:, 0:500], xs_d[0, :, 0:500])
        nc.sync.dma_start(fe[0][:, 500:1000], xs_d[0, :, 500:1000])
        nc.scalar.dma_start(fe[0][:, 1000:1500], xs_d[0, :, 1000:1500])
        nc.gpsimd.dma_start(fe[0][:, 1500:2000], xs_d[0, :, 1500:2000])
        nc.sync.dma_start(l1s[0][:], l1_d[:, 0:512])
        nc.scalar.dma_start(b0s[:], b0_d[:])
        nc.gpsimd.dma_start(b1s[:], b1_d[:])
        nc.gpsimd.dma_start(b2s[:], b2_d[:])

        # The tiny tensor_copy reads create an arrival dependency on every
        # column chunk of the previous image; gpsimd's in-order queue then
        # holds the next image's dma issues behind it. (fe tiles are
        # resident, never rotated, so the extra readers are hazard-free.)
        dummy = cpool.tile([16, 4], bf16, tag="dummy", name="dummy")

        def gate_on(n):
            for a, b in FECOLS:
                nc.gpsimd.tensor_copy(dummy[:], fe[n][0:16, b - 4 : b])

        gate_on(0)
        nc.gpsimd.dma_start(l2s[:], l2_d[:])
        load_img(nc.gpsimd, 1)
        gate_on(1)
        load_img(nc.gpsimd, 2)
        gate_on(2)
        load_img(nc.gpsimd, 3)

        # ---- main loop: quads of pairs (8 instances of one image) ----
        for qd in range(4):
            img = qd
            for hh in range(SPI // FD):
                base = hh * FD
                psq = psqpool.tile([128, FD], f32, tag="psq", name="psq")
                h2s = []
                for j in range(4):
                    p = 4 * qd + j
                    w0 = l0s[img][:, j * 128 : (j + 1) * 128]
                    w1 = l1s[img][:, j * 128 : (j + 1) * 128]
                    ps0 = pspool.tile([128, FD], f32, tag="ps", name="ps0")
                    for off, sz in SPLITS:
                        nc.tensor.matmul(
                            ps0[:, off : off + sz],
                            w0,
                            fe[img][:, base + off : base + off + sz],
                            start=True,
                            stop=True,
                        )
                    h1 = hpool.tile([128, FD], bf16, tag="h1", name="h1")
                    if j % 2 == 0:
                        nc.scalar.activation(
                            h1[:], ps0[:], AF.Relu, bias=b0s[:, p : p + 1]
                        )
                    else:
                        nc.vector.tensor_scalar(
                            h1[:], ps0[:], b0s[:, p : p + 1], 0.0, OP.add, OP.max
                        )
                    ps1 = pspool.tile([128, FD], f32, tag="ps", name="ps1")
                    for off, sz in SPLITS:
                        nc.tensor.matmul(
                            ps1[:, off : off + sz],
                            w1,
                            h1[:, off : off + sz],
                            start=True,
                            stop=True,
                        )
                    h2 = hpool.tile([128, FD], bf16, tag="h2", name="h2", bufs=6)
                    if j % 2 == 0:
                        nc.vector.tensor_scalar(
                            h2[:], ps1[:], b1s[:, p : p + 1], 0.0, OP.add, OP.max
                        )
                    else:
                        nc.scalar.activation(
                            h2[:], ps1[:], AF.Relu, bias=b1s[:, p : p + 1]
                        )
                    h2s.append(h2)
                # Filler matmuls (zero data, stomping psq before its real
                # start=True writes): they pad PE work per chunk to just
                # above the move-engine pace, so the PE queue always has
                # ready work and stays ~100% busy. That holds the HAM clock
                # gate at K=8/8 (2.4 GHz) — a 75-80%-busy stream measurably
                # drops to 1.2 GHz and never recovers, which costs far more
                # than the ~0.8us/chunk of filler.
                for _ in range(2):
                    nc.tensor.matmul(
                        psq[:, 0:512], wsrc[:, 0:128], wsrc[:, 128:640],
                        start=True, stop=True,
                    )
                # layer2 for all 4 pairs, grouped back-to-back: the four
                # matmuls per split target distinct 32-col groups of the PE
                # array (tile_position) and execute concurrently.
                for off, sz in SPLITS:
                    for j in range(4):
                        p = 4 * qd + j
                        w2 = l2s[:, 32 * p : 32 * p + 32]
                        nc.tensor.matmul(
                            psq[32 * j : 32 * j + 32, off : off + sz],
                            w2,
                            h2s[j][:, off : off + sz],
                            start=True,
                            stop=True,
                            tile_position=(0, 32 * j),
                        )
                # fused bias+move for the (quad, half) on ACT (it is the
                # faster mover; this keeps ACT:DVE at 5:4 moves per chunk).
                # Dedicated oq tiles (no pool reuse) let the output DMAs
                # drain lazily without write-after-read hazards.
                oq = cpool.tile([128, FD], f32, tag=f"oq{qd}_{hh}", name=f"oq{qd}_{hh}")
                nc.scalar.activation(
                    oq[:], psq[:], AF.Identity, bias=b2s[:, qd : qd + 1]
                )
                # contiguous 2-row reads (strided-partition reads miss deps
                # in the tile tracker and raced the move op). All on sync:
                # it is idle once the input issues drain, and keeping
                # gpsimd's software-DGE queues untouched late in the run
                # avoids a ~2us queue-drain stall in the epilogue.
                for j in range(4):
                    nc.sync.dma_start(
                        out_d[qd, j, :, base : base + FD], oq[32 * j : 32 * j + 2, :]
                    )

    nc.compile()
    _dedupe_ldweights(nc, mybir)
    return nc


def _dedupe_ldweights(nc, mybir):
    """Drop redundant PE LDWEIGHTS after compile.

    Tile emits one LDWEIGHTS per matmul; consecutive matmuls often share one
    stationary operand (split-column pairs, the warmup burst). The PE array
    is 16 independent 32x32 subarrays, so a narrow (<=32-col) load only
    clobbers its own col group: track the last-retained signature per col
    group and clear everything on a full-width load. Safe removal criteria:
    identical weights AP + tile_position as the last retained LDWEIGHTS for
    that col group, and no semaphore waits/updates on the dropped
    instruction, so synchronization is untouched.
    """
    dropped = 0
    for fn in nc.m.functions:
        for blk in fn.blocks:
            new = []
            last = {}  # col_grp -> sig
            for i in blk.instructions:
                if (
                    isinstance(i, mybir.InstLdweights)
                    and i.engine == mybir.EngineType.PE
                ):
                    tp = tuple(i.tile_position or ())
                    sig = (str(i.ins[0]), tp, i.perf_mode, i.is_transpose)
                    si = i.sync_info
                    clean = si is None or (not si.on_wait and not si.on_update)
                    try:
                        ncols = i.ins[0].free_size()
                    except Exception:
                        ncols = 128
                    key = tp[1] if len(tp) == 2 else 0
                    if ncols > 32:
                        # full/wide load touches multiple col groups
                        if clean and last.get("wide") == sig and len(last) == 1:
                            dropped += 1
                            continue
                        last = {"wide": sig}
                    else:
                        if clean and last.get(key) == sig:
                            dropped += 1
                            continue
                        last.pop("wide", None)
                        last[key] = sig
                new.append(i)
            if dropped:
                blk.instructions.clear()
                blk.instructions.extend(new)
    return dropped


def _pack_params(mask_head_params):
    """Split generated params and build the pair-packed device layouts."""
    p = np.ascontiguousarray(mask_head_params, dtype=np.float32)
    o0, o1, o2 = _W0N, _W0N + _W1N, _W0N + _W1N + _W2N
    w0 = p[:, :o0].reshape(M, C, CH)
    w1 = p[:, o0:o1].reshape(M, C, C)
    w2 = p[:, o1:o2].reshape(M, C)
    b0 = p[:, o2 : o2 + C]
    b1 = p[:, o2 + C : o2 + 2 * C]
    b2 = p[:, o2 + 2 * C :] + np.float32(_B2_SHIFT)

    import ml_dtypes as _mld

    # layer0 lhsT [66, 16*128]: rows 0-63 = x-channel weights, 64-65 = coord
    # weights; cols pair-major then (inst a | inst b).
    w0T = np.transpose(w0, (2, 0, 1))              # [66(cin), 32, 64]
    w0T = np.concatenate([w0T[2:], w0T[:2]], 0)    # x channels first, coords last
    l0t = np.ascontiguousarray(w0T.reshape(CH, M * C).astype(_mld.bfloat16))

    l1 = np.zeros((PAIRS, 128, 128), dtype=np.float32)
    l1[:, :C, :C] = np.transpose(w1[0::2], (0, 2, 1))
    l1[:, C:, C:] = np.transpose(w1[1::2], (0, 2, 1))
    l1t = np.ascontiguousarray(
        np.transpose(l1, (1, 0, 2)).reshape(128, PAIRS * 128).astype(_mld.bfloat16)
    )

    l2 = np.zeros((PAIRS, 128, 32), dtype=np.float32)
    l2[:, :C, 0] = w2[0::2]
    l2[:, C:, 1] = w2[1::2]
    l2t = np.ascontiguousarray(
        np.transpose(l2, (1, 0, 2)).reshape(128, PAIRS * 32).astype(_mld.bfloat16)
    )

    b0t = np.ascontiguousarray(np.concatenate([b0[0::2], b0[1::2]], 1).T)  # [128,16]
    b1t = np.ascontiguousarray(np.concatenate([b1[0::2], b1[1::2]], 1).T)
    # b2 packed to match the quad PSUM layout: rows 32j+r of col q hold
    # instance 8q + 2j + r.
    b2q = np.zeros((128, 4), dtype=np.float32)
    for qq in range(4):
        for j in range(4):
            b2q[32 * j, qq] = b2[8 * qq + 2 * j, 0]
            b2q[32 * j + 1, qq] = b2[8 * qq + 2 * j + 1, 0]
    return l0t, l1t, l2t, b0t, b1t, b2q


def _run(x, mask_head_params, trace=False, trace_kwargs=None):
    from concourse.bass_utils import run_bass_kernel_spmd

    if "nc" not in _COMPILED:
        _COMPILED["nc"] = _build_program()
    nc = _COMPILED["nc"]

    x = np.ascontiguousarray(x, dtype=np.float32)
    l0t, l1t, l2t, b0t, b1t, b2q = _pack_params(mask_head_params)

    xx = np.tile(np.arange(W, dtype=np.float32) / W, HPC)  # [2000]
    in_maps = []
    for k in range(N_CORES):
        h0 = k * HPC
        yy = np.repeat((h0 + np.arange(HPC, dtype=np.float32)) / W, W)
        coords = np.stack([xx, yy], 0)  # [2, 2000]
        import ml_dtypes as _mld

        xsl = x[:, :, h0 : h0 + HPC, :].reshape(N_IMG, C, SPI)
        xs = np.ascontiguousarray(
            np.concatenate(
                [xsl, np.broadcast_to(coords, (N_IMG, 2, SPI))], axis=1
            ).astype(_mld.bfloat16)
        )
        in_maps.append(
            {
                "xs": xs,
                "l0t": l0t,
                "l1t": l1t,
                "l2t": l2t,
                "b0t": b0t,
                "b1t": b1t,
                "b2q": b2q,
            }
        )

    res = run_bass_kernel_spmd(
        nc,
        in_maps,
        list(range(N_CORES)),
        trace=trace,
        **(trace_kwargs or {}),
    )

    out = np.empty((1, M, H, W), dtype=np.float32)
    for k in range(N_CORES):
        oc = res.results[k]["out"].reshape(M, HPC, W)
        out[0, :, k * HPC : (k + 1) * HPC, :] = oc
    return out, res


def kernel(x, mask_head_params, num_ins):
    n_ins = int(np.asarray(num_ins))
    assert n_ins == NUM_INS, f"kernel hardcoded for num_ins={NUM_INS}, got {n_ins}"
    out, _ = _run(x, mask_head_params)
    return out


# revision 13
# speedup vs baseline: 1.5490x; 1.3937x over previous
"""Trainium2 Bass kernel for CondLaneRNNHead-style dynamic mask head.

Computation (see reference): per-instance 3-layer 1x1-conv MLP over
per-image feature maps augmented with 2 coordinate channels.

  out[m] = w2[m] @ relu(w1[m] @ relu(w0[m] @ [coords; x[img(m)]] + b0[m]) + b1[m]) + b2[m]

Shapes: x [4, 64, 80, 200] f32, mask_head_params [32, 8513] f32, num_ins=8.
Output [1, 32, 80, 200] f32.

Sharding: spatial, along H. Core k processes rows [10k, 10k+10) of all 4
images for all 32 instances. This replicates only the (small) per-instance
params across cores; the big x tensor is read exactly once in aggregate.

Device kernel structure (per core):
  - feats[img] SBUF tile [66, 2000]: partitions 0-63 = x channels,
    partitions 64-65 = (xx/W, yy/W) coordinate rows. Layer0 matmuls run
    K=66 directly (no zero-pad): padding K to 128 only added DMA bytes and
    required GpSimd memsets that delayed the input DMA head.
  - Instances are packed in PAIRS (2 instances of the same image):
      layer0: lhsT [66, 128]  (cols 0-63 inst a, 64-127 inst b), one matmul
              computes both instances' 64 hidden channels.
      layer1: lhsT [128, 128] block-diagonal (w1a.T | w1b.T).
      layer2: lhsT [128, 32]  cols 0-1 used ([w2a;0] | [0;w2b]), 30 zero
              cols so the matmul initializes its full 32-partition group.
    Matmuls run in bf16 (fp32 PSUM accumulate); fp32r was measured 5x
    slower (no fast-weight-load, no background weight buffer).
  - The four layer2 matmuls of a quad (8 instances, one image) write ONE
    PSUM tile at partition offsets {0,32,64,96} via tile_position, and are
    emitted back-to-back at the end of each chunk: matmuls to distinct
    32-col groups of the PE array execute concurrently (per-subarray
    tiling), so the group costs ~1 matmul span instead of 4.
  - ReLU+bias PSUM->SBUF moves are the throughput-critical resource
    (~1.0us ACT / ~1.2us DVE per [128,1000] move, 1x perf mode since the
    source is fp32 PSUM). 9 moves/chunk split 5:4 between ACT and DVE.
  - HAM: the PE clock gate defaults to K=4/8 (1.2 GHz); ~3.4us of activity
    releases it to 2.4 GHz. A small warmup burst (8 matmuls on a zeroed
    tile) runs during the input-DMA head to pre-warm the gate; the old
    44-matmul burst serialized ~28us of PE time in front of real work.
"""

import numpy as np
from contextlib import ExitStack

N_IMG, C, H, W = 4, 64, 80, 200
NUM_INS = 8
M = N_IMG * NUM_INS          # 32 instances
N_CORES = 8
HPC = H // N_CORES           # 10 rows of H per core
SPI = HPC * W                # 2000 spatial positions per image slice
PAIRS = M // 2               # 16
CH = C + 2                   # 66 input channels incl. coords
# activation chunk: 500 cols = one PSUM bank per [128,500] f32 tile, which
# buys a 6-deep psum pool (1.5 chunks of scheduler lookahead) and single
# un-split matmuls per (pair, layer).
FD = 500

_W0N, _W1N, _W2N = CH * C, C * C, C
_B2_SHIFT = -2.19

_COMPILED = {}


def _build_program():
    import concourse.bacc as bacc
    import concourse.tile as tile
    from concourse import mybir

    dt = mybir.dt
    AF = mybir.ActivationFunctionType
    OP = mybir.AluOpType

    nc = bacc.Bacc("TRN2", target_bir_lowering=False, debug=False)

    # xs packs the 2 coordinate rows below the 64 x-channels so each image's
    # feats tile is filled by column-chunk DMAs only.
    xs_d = nc.dram_tensor("xs", [N_IMG, CH, SPI], dt.bfloat16, kind="ExternalInput").ap()
    l0_d = nc.dram_tensor("l0t", [CH, PAIRS * 128], dt.bfloat16, kind="ExternalInput").ap()
    l1_d = nc.dram_tensor("l1t", [128, PAIRS * 128], dt.bfloat16, kind="ExternalInput").ap()
    # layer2 runs in bf16: fp32r matmuls require dst start_partition == 0,
    # which the quad partition-packing (offsets 32/64/96) violates.
    l2_d = nc.dram_tensor("l2t", [128, PAIRS * 32], dt.bfloat16, kind="ExternalInput").ap()
    b0_d = nc.dram_tensor("b0t", [128, PAIRS], dt.float32, kind="ExternalInput").ap()
    b1_d = nc.dram_tensor("b1t", [128, PAIRS], dt.float32, kind="ExternalInput").ap()
    b2_d = nc.dram_tensor("b2q", [128, 4], dt.float32, kind="ExternalInput").ap()
    # out[q, j, r, :] = instance 8q + 2j + r, i.e. plain instance-major order
    out_d = nc.dram_tensor("out", [4, 4, 2, SPI], dt.float32, kind="ExternalOutput").ap()

    f32 = dt.float32
    bf16 = dt.bfloat16

    with tile.TileContext(nc) as tc, ExitStack() as ctx:
        cpool = ctx.enter_context(tc.tile_pool(name="const", bufs=1))
        hpool = ctx.enter_context(tc.tile_pool(name="work", bufs=4))
        # one PSUM bank per [128, FD=500] f32 tile; a pair's L1 matmul
        # overwrites the same tile its L0 used (the WAR on the h1 move
        # serializes them, which the data dependency forces anyway), so a
        # chunk makes only 4 allocations and 6 bufs give 1.5 chunks of
        # runnable-matmul lookahead for the PE.
        pspool = ctx.enter_context(tc.tile_pool(name="ps", bufs=6, space="PSUM"))
        psqpool = ctx.enter_context(tc.tile_pool(name="psq", bufs=2, space="PSUM"))

        # ---- PE/ACT warmup ----
        # The dummy Relu makes the ACT table-set DMA issue at t~0 instead of
        # queueing behind input DMAs. The matmul burst warms the HAM clock
        # gate (default K=4/8 = 1.2 GHz; ~3.4us of activity releases it to
        # 2.4 GHz) during the input-DMA head; the burst is sized to bridge
        # until image-0 data lands (~11us) — the matmuls WAW-serialize on
        # wps (~0.6us each) so 8 of them cover ~5us of wall clock.
        wsrc = cpool.tile([128, 640], bf16, tag="wsrc", name="wsrc")
        nc.vector.memset(wsrc[:], 0.0)
        wact = cpool.tile([128, 8], f32, tag="wact", name="wact")
        nc.scalar.activation(wact[:], wsrc[:, 0:8], AF.Relu, bias=0.0)
        wps = pspool.tile([128, FD], f32, tag="ps", name="wps")
        for _ in range(8):
            nc.tensor.matmul(
                wps[:, 0:FD], wsrc[:, 0:128], wsrc[:, 128 : 128 + FD],
                start=True, stop=True,
            )

        # ---- resident tiles + loads ----
        # DMA engines round-robin ALL queued transfers, so issuing
        # everything up front makes image 0's data finish only when the
        # whole 1.7MB input drains (~22us, measured). Instead: issue only
        # image 0+1 (and the small tensors) up front on the sync queue, and
        # gate images 2/3 behind tiny gpsimd reads of fe0/fe1 arrival so
        # their transfers cannot compete with the data the pipeline needs
        # first.
        b0s = cpool.tile([128, PAIRS], f32, tag="b0s", name="b0s")
        b1s = cpool.tile([128, PAIRS], f32, tag="b1s", name="b1s")
        b2s = cpool.tile([128, 4], f32, tag="b2s", name="b2s")

        fe = []
        l0s = []
        l1s = []
        for n in range(N_IMG):
            g0 = cpool.tile([CH, 4 * 128], bf16, tag=f"l0g{n}", name=f"l0g{n}")
            l0s.append(g0)
            fe.append(cpool.tile([CH, SPI], bf16, tag=f"fe{n}", name=f"fe{n}"))
            g1 = cpool.tile([128, 4 * 128], bf16, tag=f"l1g{n}", name=f"l1g{n}")
            l1s.append(g1)
        l2s = cpool.tile([128, PAIRS * 32], bf16, tag="l2s", name="l2s")

        FECOLS = ((0, 500), (500, 1000), (1000, 1500), (1500, 2000))

        def load_img(eng, n):
            eng.dma_start(l0s[n][:], l0_d[:, n * 512 : (n + 1) * 512])
            # column-chunk split: 4 DMAs spread over engines, and the first
            # chunk's worth of data arrives ~4x sooner than one big DMA
            for a, b in FECOLS:
                eng.dma_start(fe[n][:, a:b], xs_d[n, :, a:b])
            eng.dma_start(l1s[n][:], l1_d[:, n * 512 : (n + 1) * 512])

        # First wave split across all three issue queues (each dma_start
        # costs ~0.6us of issue time on its queue) so image 0 is fully in
        # SBUF ~5us after the preamble, right as the warmup burst ends.
        # DMA engines round-robin ALL queued transfers, so nothing else may
        # be in flight until image 0 lands: every later image is gated
        # behind the previous image's feature arrival.
        nc.sync.dma_start(l0s[0][:], l0_d[:, 0:512])
        nc.sync.dma_start(fe[0][:, 0:500], xs_d[0, :, 0:500])
        nc.sync.dma_start(fe[0][:, 500:1000], xs_d[0, :, 500:1000])
        nc.scalar.dma_start(fe[0][:, 1000:1500], xs_d[0, :, 1000:1500])
        nc.gpsimd.dma_start(fe[0][:, 1500:2000], xs_d[0, :, 1500:2000])
        nc.sync.dma_start(l1s[0][:], l1_d[:, 0:512])
        nc.scalar.dma_start(b0s[:], b0_d[:])
        nc.gpsimd.dma_start(b1s[:], b1_d[:])
        nc.gpsimd.dma_start(b2s[:], b2_d[:])

        # The tiny tensor_copy reads create an arrival dependency on every
        # column chunk of the previous image; gpsimd's in-order queue then
        # holds the next image's dma issues behind it. (fe tiles are
        # resident, never rotated, so the extra readers are hazard-free.)
        dummy = cpool.tile([16, 4], bf16, tag="dummy", name="dummy")

        def gate_on(n):
            for a, b in FECOLS:
                nc.gpsimd.tensor_copy(dummy[:], fe[n][0:16, b - 4 : b])

        gate_on(0)
        nc.gpsimd.dma_start(l2s[:], l2_d[:])
        load_img(nc.gpsimd, 1)
        gate_on(1)
        load_img(nc.gpsimd, 2)
        gate_on(2)
        load_img(nc.gpsimd, 3)

        # ---- main loop: quads of pairs (8 instances of one image) ----
        # Per chunk: the 4 pairs' L0 matmuls are emitted as one burst (all
        # independent, deepening the PE's ready queue), then the L1 burst
        # (each gated only on its own h1 move), then the 4 L2 matmuls
        # back-to-back — those target distinct 32-col groups of the PE
        # array (tile_position) and execute concurrently.
        for qd in range(4):
            img = qd
            for cc in range(SPI // FD):
                base = cc * FD
                psq = psqpool.tile([128, FD], f32, tag="psq", name="psq")
                pst = []
                h1s = []
                for j in range(4):
                    p = 4 * qd + j
                    w0 = l0s[img][:, j * 128 : (j + 1) * 128]
                    ps = pspool.tile([128, FD], f32, tag="ps", name=f"ps{j}")
                    nc.tensor.matmul(
                        ps[:], w0, fe[img][:, base : base + FD],
                        start=True, stop=True,
                    )
                    pst.append(ps)
                    h1 = hpool.tile([128, FD], bf16, tag="h1", name="h1", bufs=6)
                    if j % 2 == 0:
                        nc.scalar.activation(
                            h1[:], ps[:], AF.Relu, bias=b0s[:, p : p + 1]
                        )
                    else:
                        nc.vector.tensor_scalar(
                            h1[:], ps[:], b0s[:, p : p + 1], 0.0, OP.add, OP.max
                        )
                    h1s.append(h1)
                h2s = []
                for j in range(4):
                    p = 4 * qd + j
                    w1 = l1s[img][:, j * 128 : (j + 1) * 128]
                    nc.tensor.matmul(
                        pst[j][:], w1, h1s[j][:], start=True, stop=True,
                    )
                    h2 = hpool.tile([128, FD], bf16, tag="h2", name="h2", bufs=6)
                    if j % 2 == 0:
                        nc.vector.tensor_scalar(
                            h2[:], pst[j][:], b1s[:, p : p + 1], 0.0, OP.add, OP.max
                        )
                    else:
                        nc.scalar.activation(
                            h2[:], pst[j][:], AF.Relu, bias=b1s[:, p : p + 1]
                        )
                    h2s.append(h2)
                for j in range(4):
                    p = 4 * qd + j
                    w2 = l2s[:, 32 * p : 32 * p + 32]
                    nc.tensor.matmul(
                        psq[32 * j : 32 * j + 32, :],
                        w2,
                        h2s[j][:],
                        start=True,
                        stop=True,
                        tile_position=(0, 32 * j),
                    )
                # fused bias+move for the (quad, chunk) on ACT (it is the
                # faster mover; this keeps ACT:DVE at 5:4 moves per chunk).
                # Dedicated oq tiles (no pool reuse) let the output DMAs
                # drain lazily without write-after-read hazards.
                oq = cpool.tile([128, FD], f32, tag=f"oq{qd}_{cc}", name=f"oq{qd}_{cc}")
                nc.scalar.activation(
                    oq[:], psq[:], AF.Identity, bias=b2s[:, qd : qd + 1]
                )
                # contiguous 2-row reads (strided-partition reads miss deps
                # in the tile tracker and raced the move op). All on sync:
                # it is idle once the input issues drain, and keeping
                # gpsimd's software-DGE queues untouched late in the run
                # avoids a ~2us queue-drain stall in the epilogue.
                for j in range(4):
                    nc.sync.dma_start(
                        out_d[qd, j, :, base : base + FD], oq[32 * j : 32 * j + 2, :]
                    )

    nc.compile()
    _dedupe_ldweights(nc, mybir)
    return nc


def _dedupe_ldweights(nc, mybir):
    """Drop redundant PE LDWEIGHTS after compile.

    Tile emits one LDWEIGHTS per matmul; consecutive matmuls often share one
    stationary operand (split-column pairs, the warmup burst). The PE array
    is 16 independent 32x32 subarrays, so a narrow (<=32-col) load only
    clobbers its own col group: track the last-retained signature per col
    group and clear everything on a full-width load. Safe removal criteria:
    identical weights AP + tile_position as the last retained LDWEIGHTS for
    that col group, and no semaphore waits/updates on the dropped
    instruction, so synchronization is untouched.
    """
    dropped = 0
    for fn in nc.m.functions:
        for blk in fn.blocks:
            new = []
            last = {}  # col_grp -> sig
            for i in blk.instructions:
                if (
                    isinstance(i, mybir.InstLdweights)
                    and i.engine == mybir.EngineType.PE
                ):
                    tp = tuple(i.tile_position or ())
                    sig = (str(i.ins[0]), tp, i.perf_mode, i.is_transpose)
                    si = i.sync_info
                    clean = si is None or (not si.on_wait and not si.on_update)
                    try:
                        ncols = i.ins[0].free_size()
                    except Exception:
                        ncols = 128
                    key = tp[1] if len(tp) == 2 else 0
                    if ncols > 32:
                        # full/wide load touches multiple col groups
                        if clean and last.get("wide") == sig and len(last) == 1:
                            dropped += 1
                            continue
                        last = {"wide": sig}
                    else:
                        if clean and last.get(key) == sig:
                            dropped += 1
                            continue
                        last.pop("wide", None)
                        last[key] = sig
                new.append(i)
            if dropped:
                blk.instructions.clear()
                blk.instructions.extend(new)
    return dropped


def _pack_params(mask_head_params):
    """Split generated params and build the pair-packed device layouts."""
    p = np.ascontiguousarray(mask_head_params, dtype=np.float32)
    o0, o1, o2 = _W0N, _W0N + _W1N, _W0N + _W1N + _W2N
    w0 = p[:, :o0].reshape(M, C, CH)
    w1 = p[:, o0:o1].reshape(M, C, C)
    w2 = p[:, o1:o2].reshape(M, C)
    b0 = p[:, o2 : o2 + C]
    b1 = p[:, o2 + C : o2 + 2 * C]
    b2 = p[:, o2 + 2 * C :] + np.float32(_B2_SHIFT)

    import ml_dtypes as _mld

    # layer0 lhsT [66, 16*128]: rows 0-63 = x-channel weights, 64-65 = coord
    # weights; cols pair-major then (inst a | inst b).
    w0T = np.transpose(w0, (2, 0, 1))              # [66(cin), 32, 64]
    w0T = np.concatenate([w0T[2:], w0T[:2]], 0)    # x channels first, coords last
    l0t = np.ascontiguousarray(w0T.reshape(CH, M * C).astype(_mld.bfloat16))

    l1 = np.zeros((PAIRS, 128, 128), dtype=np.float32)
    l1[:, :C, :C] = np.transpose(w1[0::2], (0, 2, 1))
    l1[:, C:, C:] = np.transpose(w1[1::2], (0, 2, 1))
    l1t = np.ascontiguousarray(
        np.transpose(l1, (1, 0, 2)).reshape(128, PAIRS * 128).astype(_mld.bfloat16)
    )

    l2 = np.zeros((PAIRS, 128, 32), dtype=np.float32)
    l2[:, :C, 0] = w2[0::2]
    l2[:, C:, 1] = w2[1::2]
    l2t = np.ascontiguousarray(
        np.transpose(l2, (1, 0, 2)).reshape(128, PAIRS * 32).astype(_mld.bfloat16)
    )

    b0t = np.ascontiguousarray(np.concatenate([b0[0::2], b0[1::2]], 1).T)  # [128,16]
    b1t = np.ascontiguousarray(np.concatenate([b1[0::2], b1[1::2]], 1).T)
    # b2 packed to match the quad PSUM layout: rows 32j+r of col q hold
    # instance 8q + 2j + r.
    b2q = np.zeros((128, 4), dtype=np.float32)
    for qq in range(4):
        for j in range(4):
            b2q[32 * j, qq] = b2[8 * qq + 2 * j, 0]
            b2q[32 * j + 1, qq] = b2[8 * qq + 2 * j + 1, 0]
    return l0t, l1t, l2t, b0t, b1t, b2q


def _run(x, mask_head_params, trace=False, trace_kwargs=None):
    from concourse.bass_utils import run_bass_kernel_spmd

    if "nc" not in _COMPILED:
        _COMPILED["nc"] = _build_program()
    nc = _COMPILED["nc"]

    x = np.ascontiguousarray(x, dtype=np.float32)
    l0t, l1t, l2t, b0t, b1t, b2q = _pack_params(mask_head_params)

    xx = np.tile(np.arange(W, dtype=np.float32) / W, HPC)  # [2000]
    in_maps = []
    for k in range(N_CORES):
        h0 = k * HPC
        yy = np.repeat((h0 + np.arange(HPC, dtype=np.float32)) / W, W)
        coords = np.stack([xx, yy], 0)  # [2, 2000]
        import ml_dtypes as _mld

        xsl = x[:, :, h0 : h0 + HPC, :].reshape(N_IMG, C, SPI)
        xs = np.ascontiguousarray(
            np.concatenate(
                [xsl, np.broadcast_to(coords, (N_IMG, 2, SPI))], axis=1
            ).astype(_mld.bfloat16)
        )
        in_maps.append(
            {
                "xs": xs,
                "l0t": l0t,
                "l1t": l1t,
                "l2t": l2t,
                "b0t": b0t,
                "b1t": b1t,
                "b2q": b2q,
            }
        )

    res = run_bass_kernel_spmd(
        nc,
        in_maps,
        list(range(N_CORES)),
        trace=trace,
        **(trace_kwargs or {}),
    )

    out = np.empty((1, M, H, W), dtype=np.float32)
    for k in range(N_CORES):
        oc = res.results[k]["out"].reshape(M, HPC, W)
        out[0, :, k * HPC : (k + 1) * HPC, :] = oc
    return out, res


def kernel(x, mask_head_params, num_ins):
    n_ins = int(np.asarray(num_ins))
    assert n_ins == NUM_INS, f"kernel hardcoded for num_ins={NUM_INS}, got {n_ins}"
    out, _ = _run(x, mask_head_params)
    return out
